# revision 17
# baseline (speedup 1.0000x reference)
"""Trainium2 Bass kernel for nn_Loss_65781719105930 (YOLO-style detection loss).

Strategy (pure data parallelism, 8 cores, 32 images each):
  host:   replicate the reference's target-build scatter (small int64 inputs),
          compact occupied cells, pre-pack aux tables + prediction columns into
          three contiguous DMA payloads; gather the target-class logit per
          (cell, anchor) host-side.
  device: dense pass over the 5 conf channels (sum of sigmoid^2), plus IoU /
          first-argmax / best-anchor-select / cross-entropy on compacted tiles.

Numeric tricks that keep the scalar engine on ONE activation-table set
(exp_and_others = {tanh, exp, square}):
  sigmoid(x)   = (1 + tanh(x/2)) / 2      -> work in xi = 2x-1 coords, the
                                             0.5 factors fold into host consts
  sqrt(exp(x)*anchor) = exp(x/2)*sqrt(anchor)
  ln(x)        ~ bitcast_i32(x) * ln2/2^23 - 126.94269504*ln2   (abs err ~2e-2
                 worst, mean-centered; loss tolerance is 2e-2 relative)

The grid offset cancels algebraically in both the IoU and the box loss, so it
never appears on device.
"""
import numpy as np

# ---------------------------------------------------------------- constants
NCLS = 20
H = W = 32
HWC = H * W            # 1024 cells/image
A = 5
M = 50
B = 256
CORES = 8
BC = B // CORES        # 32 images per core
CH = A * (5 + NCLS)    # 125 channels
P = 128
LAM_COORD, LAM_OBJ, LAM_NOOBJ, LAM_CLS = 5.0, 1.0, 0.5, 1.0

LN2 = float(np.log(2.0))
LOG_BIAS = 126.94269504   # mean-centering constant for the log2 bit trick

_CACHE = {}


def _bf16(x):
    """float32 ndarray -> ml_dtypes.bfloat16 (RNE)."""
    import ml_dtypes
    return np.asarray(x, dtype=np.float32).astype(ml_dtypes.bfloat16)


# ---------------------------------------------------------------- host prep
def _build_target_np(gt_boxes, gt_classes, num_box):
    """Numpy replication of reference.build_target (last object wins, first-max
    class argmax). Returns per-cell [B, HWC] arrays."""
    Bn = gt_boxes.shape[0]
    valid = np.arange(M)[None, :] < num_box[:, None]
    x = gt_boxes[..., 0].astype(np.float32) * H
    y = gt_boxes[..., 1].astype(np.float32) * H
    gx = np.floor(x).astype(np.int64)
    gy = np.floor(y).astype(np.int64)
    flat = np.where(valid, gy * W + gx, HWC)
    bi = np.broadcast_to(np.arange(Bn)[:, None], (Bn, M))

    vals = np.stack([np.ones_like(x), x - gx, y - gy,
                     gt_boxes[..., 2].astype(np.float32) * H,
                     gt_boxes[..., 3].astype(np.float32) * H], axis=-1)
    tgt_box = np.zeros((Bn, HWC + 1, 5), dtype=np.float32)
    tgt_box[bi, flat] = vals
    tgt_cls = np.zeros((Bn, HWC + 1, NCLS), dtype=np.float32)
    tgt_cls[bi, flat, gt_classes.astype(np.int64)] = 1.0

    tgt_box = tgt_box[:, :HWC]
    obj = tgt_box[..., 0]
    cls_t = np.argmax(tgt_cls[:, :HWC], axis=-1).astype(np.int64)
    return obj, tgt_box[..., 1], tgt_box[..., 2], tgt_box[..., 3], tgt_box[..., 4], cls_t


def _split_multi_waits(nc):
    """This container's walrus accepts only ONE sem-wait per instruction; hoist
    extra waits onto standalone NoOps."""
    import concourse.mybir as mybir
    import bass_rust
    n = 0
    for fn in nc.m.functions:
        for blk in fn.blocks:
            new = []
            for ins in blk.instructions:
                si = ins.sync_info
                waits = list(si.on_wait) if si is not None else []
                if len(waits) > 1:
                    for w in waits[:-1]:
                        nop = mybir.InstNoOp(name=f"{ins.name}-w{n}")
                        nop.engine = ins.engine
                        nop.sync_info = bass_rust.SyncInfo(on_wait=[w], on_update=[])
                        new.append(nop)
                        n += 1
                    si.on_wait = [waits[-1]]
                    ins.sync_info = si
                new.append(ins)
            blk.instructions = new
    return n


# ---------------------------------------------------------------- bass build
def _build_nc(T, split=True):
    """Build the per-core kernel for T cell-blocks per partition (P*T slots).

    SBUF layouts (all [128, n], f32 unless noted):
      fpack [P, 25T + 18+18+T+5+10 + 5T + 4T+4T+T]:
        cols_xw   (t,a,{conf,x,y,w,h})      25T   occupied-cell chans 20..24
        B1        (t,{x,y})                 2T    xi-space target box lo edges
        B2        (t,{x,y})                 2T    xi-space target box hi edges
        TAREA     (t)                       T     tw*th (physical, cell units)
        WCONST    (a)                       5     A - a   (first-argmax tiebreak)
        SQA       (a,{w,h})                 10    sqrt(anchor)
        S_AUX     (t,a)                     5T    target-class logit
        AUX4      (q,t) q in {x,y,w,h}      4T    (2xo-1, 2yo-1, sqrt tw, sqrt th)
        OSCL4     (q,t)                     4T    obj * {.25,.25,1,1}
        OBJ       (t)                       T     obj mask
      lgpack bf16 [P, 100T]: logits (t,a,j)
      confd  bf16 [P, 1280]: all conf channels of all cells (dense noobj pass)
      partials out [P, 8]: 0 box, 1 sum obj*u^2, 2 sum obj*u (u = tanh(conf/2)),
        3 sum obj*ce, 4 dense sum sigmoid(conf)^2
    """
    import concourse.bass as bass
    import concourse.mybir as mybir
    import concourse.tile as tile

    f32 = mybir.dt.float32
    bf16 = mybir.dt.bfloat16
    i32 = mybir.dt.int32
    AF = mybir.ActivationFunctionType
    OP = mybir.AluOpType
    AX = mybir.AxisListType

    TA = T * A           # (t, a) flat size
    TA2 = TA * 2
    NF = 25 * T + (2 * T + 2 * T + T + 5 + 10) + 5 * T + 4 * T + 4 * T + T

    # fpack free-dim offsets
    O_XW = 0
    O_B1 = 25 * T
    O_B2 = O_B1 + 2 * T
    O_TAREA = O_B2 + 2 * T
    O_WCONST = O_TAREA + T
    O_SQA = O_WCONST + 5
    O_SAUX = O_SQA + 10
    O_AUX4 = O_SAUX + 5 * T
    O_OSCL4 = O_AUX4 + 4 * T
    O_OBJ = O_OSCL4 + 4 * T
    assert O_OBJ + T == NF

    def _v(ap, off, dims):
        """Sub-view of a tile AP: keep its partition dim, replace free dims."""
        return bass.AP(tensor=ap.tensor, offset=ap.offset + off,
                       ap=[list(ap.ap[0])] + dims)

    import os as _os
    debug = _os.environ.get("K_DEBUG", "0") == "1"
    nc = bass.Bass("TRN2")
    fpack_d = nc.declare_dram_parameter("fpack", [P, NF], f32, isOutput=False)
    lgpack_d = nc.declare_dram_parameter("lgpack", [P, 100 * T], bf16, isOutput=False)
    confd_d = nc.declare_dram_parameter("confd", [P, BC * A * HWC // P], bf16,
                                        isOutput=False)
    partials_d = nc.declare_dram_parameter("partials", [P, 8], f32, isOutput=True)
    if debug:
        dbg_d = nc.declare_dram_parameter("dbg", [P, 4 * TA], f32, isOutput=True)

    DF = BC * A * HWC // P   # 1280 dense conf elements per partition

    with tile.TileContext(nc) as tc:
        with tc.tile_pool(name="sb", bufs=1) as pool:
            # ---------------- input DMAs, priority order, all on sync HWDGE
            fp = pool.tile([P, NF], f32, name="fp")
            nc.sync.dma_start(out=fp[:], in_=fpack_d[:])
            lg_in = pool.tile([P, 100 * T], bf16, name="lg_in")
            nc.sync.dma_start(out=lg_in[:], in_=lgpack_d[:])
            confd = pool.tile([P, DF], bf16, name="confd")
            nc.sync.dma_start(out=confd[:], in_=confd_d[:])

            partials = pool.tile([P, 8], f32, name="partials")

            # ---------------- scalar engine program (one act-table set)
            # SRC: (q, t, a) with q in {u, x, y, w, h, ce}; best-anchor
            # selection later works on all six quantities in one mul+reduce.
            SRC = pool.tile([P, 6 * TA], f32, name="SRC")

            # u, xi_x, xi_y = tanh(chan{conf,x,y}/2); chans 0..2 of cols_xw
            nc.scalar.activation(
                _v(SRC[:], 0, [[TA, 3], [A, T], [1, A]]),
                _v(fp[:], O_XW, [[5, 3], [25, T], [1, A]]),
                AF.Tanh, scale=0.5)
            # EW = exp(chan{w,h}/2)
            EW = pool.tile([P, TA2], f32, name="EW")
            nc.scalar.activation(
                _v(EW[:], 0, [[1, 2], [2 * A, T], [2, A]]),
                _v(fp[:], O_XW + 15, [[5, 2], [25, T], [1, A]]),
                AF.Exp, scale=0.5)
            # e = exp(logits), bf16, (t, a, j)
            e = pool.tile([P, 100 * T], bf16, name="e")
            nc.scalar.activation(
                _v(e[:], 0, [[100, T], [NCLS, A], [1, NCLS]]),
                _v(lg_in[:], 0, [[100, T], [NCLS, A], [1, NCLS]]),
                AF.Exp)
            # dense noobj pass: ud = tanh(c/2); sum sigma^2 = 0.25*(N + 2*sum u
            # + sum u^2) -- avoids a bias const AP on the Square activation.
            UD = pool.tile([P, DF], f32, name="UD")
            nc.scalar.activation(UD[:], confd[:], AF.Tanh, scale=0.5,
                                 accum_out=_v(partials[:], 5, [[1, 1]]))
            SQD = pool.tile([P, DF], f32, name="SQD")
            nc.scalar.activation(SQD[:], UD[:], AF.Square,
                                 accum_out=_v(partials[:], 4, [[1, 1]]))

            # ---------------- vector engine program
            tcnt = [0]

            def tmp(n):
                tcnt[0] += 1
                return pool.tile([P, n], f32, name=f"t{tcnt[0]}")

            # sh = EW * sqrt(anchor)  (= sqrt(pred_wh)); into SRC q3,q4
            SH = _v(SRC[:], 3 * TA, [[TA, 2], [A, T], [1, A]])
            nc.vector.tensor_tensor(
                out=SH,
                in0=_v(EW[:], 0, [[1, 2], [2 * A, T], [2, A]]),
                in1=_v(fp[:], O_SQA, [[1, 2], [0, T], [2, A]]),
                op=OP.mult)
            # wfull = sh*sh = pred_wh (xi-space half-width), (d,t,a) [P, 2TA]
            wf = tmp(TA2)
            SH2 = _v(SRC[:], 3 * TA, [[TA, 2], [1, TA]])
            WF = _v(wf[:], 0, [[TA, 2], [1, TA]])
            nc.vector.tensor_tensor(out=WF, in0=SH2, in1=SH2, op=OP.mult)

            # IoU in xi coords. XY = SRC q1,q2 as (d,t,a)
            XY = _v(SRC[:], TA, [[TA, 2], [1, TA]])
            lo = tmp(TA2)
            nc.vector.tensor_tensor(out=lo[:], in0=XY, in1=WF, op=OP.subtract)
            hi = tmp(TA2)
            nc.vector.tensor_tensor(out=hi[:], in0=XY, in1=WF, op=OP.add)
            # target edges, (d,t) broadcast over a -> (d,t,a)
            B1v = _v(fp[:], O_B1, [[T, 2], [1, T], [0, A]])
            B2v = _v(fp[:], O_B2, [[T, 2], [1, T], [0, A]])
            LOv = _v(lo[:], 0, [[TA, 2], [A, T], [1, A]])
            HIv = _v(hi[:], 0, [[TA, 2], [A, T], [1, A]])
            t1 = tmp(TA2)
            nc.vector.tensor_tensor(out=_v(t1[:], 0, [[TA, 2], [A, T], [1, A]]),
                                    in0=HIv, in1=B2v, op=OP.min)
            t2 = tmp(TA2)
            nc.vector.tensor_tensor(out=_v(t2[:], 0, [[TA, 2], [A, T], [1, A]]),
                                    in0=LOv, in1=B1v, op=OP.max)
            t3 = tmp(TA2)
            nc.vector.tensor_tensor(out=t3[:], in0=t1[:], in1=t2[:], op=OP.subtract)
            # iw = max(t3, 0) * 0.5 -> physical overlap widths (d,t,a)
            iwih = tmp(TA2)
            nc.vector.tensor_scalar(out=iwih[:], in0=t3[:], scalar1=0.0,
                                    scalar2=0.5, op0=OP.max, op1=OP.mult)
            inter = tmp(TA)
            nc.vector.tensor_tensor(out=inter[:], in0=_v(iwih[:], 0, [[1, TA]]),
                                    in1=_v(iwih[:], TA, [[1, TA]]), op=OP.mult)
            areaA = tmp(TA)
            nc.vector.tensor_tensor(out=areaA[:], in0=_v(wf[:], 0, [[1, TA]]),
                                    in1=_v(wf[:], TA, [[1, TA]]), op=OP.mult)
            u1 = tmp(TA)
            nc.vector.tensor_tensor(out=_v(u1[:], 0, [[A, T], [1, A]]),
                                    in0=_v(areaA[:], 0, [[A, T], [1, A]]),
                                    in1=_v(fp[:], O_TAREA, [[1, T], [0, A]]),
                                    op=OP.add)
            u2 = tmp(TA)
            nc.vector.tensor_tensor(out=u2[:], in0=u1[:], in1=inter[:],
                                    op=OP.subtract)
            # this container's walrus rejects ISA-level DVE ops (custom
            # reciprocal_approx_fast / tensor_tensor_reduce): "ISA wrong length"
            rcp = tmp(TA)
            nc.vector.reciprocal(out=rcp[:], in_=u2[:])
            iou = tmp(TA)
            nc.vector.tensor_tensor(out=iou[:], in0=inter[:], in1=rcp[:],
                                    op=OP.mult)

            # first-argmax over a -> fmask (exact float equality + tiebreak)
            rmax = tmp(T)
            nc.vector.tensor_reduce(out=rmax[:],
                                    in_=_v(iou[:], 0, [[A, T], [1, A]]),
                                    axis=AX.X, op=OP.max)
            eq = tmp(TA)
            nc.vector.tensor_tensor(out=_v(eq[:], 0, [[A, T], [1, A]]),
                                    in0=_v(iou[:], 0, [[A, T], [1, A]]),
                                    in1=_v(rmax[:], 0, [[1, T], [0, A]]),
                                    op=OP.is_equal)
            fval = tmp(TA)
            nc.vector.tensor_tensor(out=_v(fval[:], 0, [[A, T], [1, A]]),
                                    in0=_v(eq[:], 0, [[A, T], [1, A]]),
                                    in1=_v(fp[:], O_WCONST, [[0, T], [1, A]]),
                                    op=OP.mult)
            m2 = tmp(T)
            nc.vector.tensor_reduce(out=m2[:],
                                    in_=_v(fval[:], 0, [[A, T], [1, A]]),
                                    axis=AX.X, op=OP.max)
            fmask = tmp(TA)
            nc.vector.tensor_tensor(out=_v(fmask[:], 0, [[A, T], [1, A]]),
                                    in0=_v(fval[:], 0, [[A, T], [1, A]]),
                                    in1=_v(m2[:], 0, [[1, T], [0, A]]),
                                    op=OP.is_equal)

            # cls: se = sum_j e, lse via log2 bit trick, ce = lse - s  (q5)
            se = tmp(TA)
            nc.vector.tensor_reduce(out=_v(se[:], 0, [[A, T], [1, A]]),
                                    in_=_v(e[:], 0, [[100, T], [NCLS, A], [1, NCLS]]),
                                    axis=AX.X, op=OP.add)
            lgf = tmp(TA)
            nc.vector.tensor_copy(out=lgf[:], in_=se[:].bitcast(i32))
            lse = tmp(TA)
            nc.vector.tensor_scalar(out=lse[:], in0=lgf[:],
                                    scalar1=LN2 / (1 << 23),
                                    scalar2=-LOG_BIAS * LN2,
                                    op0=OP.mult, op1=OP.add)
            nc.vector.tensor_tensor(out=_v(SRC[:], 5 * TA, [[A, T], [1, A]]),
                                    in0=_v(lse[:], 0, [[A, T], [1, A]]),
                                    in1=_v(fp[:], O_SAUX, [[A, T], [1, A]]),
                                    op=OP.subtract)

            # best-anchor selection of all six quantities in one mul+reduce
            selm = pool.tile([P, 6 * TA], f32, name="selm")
            nc.vector.tensor_tensor(out=_v(selm[:], 0, [[TA, 6], [1, TA]]),
                                    in0=_v(SRC[:], 0, [[TA, 6], [1, TA]]),
                                    in1=_v(fmask[:], 0, [[0, 6], [1, TA]]),
                                    op=OP.mult)
            selq = pool.tile([P, 6 * T], f32, name="selq")
            nc.vector.tensor_reduce(out=_v(selq[:], 0, [[T, 6], [1, T]]),
                                    in_=_v(selm[:], 0, [[TA, 6], [A, T], [1, A]]),
                                    axis=AX.X, op=OP.add)

            def ttr(in0, in1, col, n):
                junk = tmp(n)
                nc.vector.tensor_tensor(out=junk[:], in0=in0, in1=in1,
                                        op=OP.mult)
                nc.vector.tensor_reduce(out=_v(partials[:], col, [[1, 1]]),
                                        in_=junk[:], axis=AX.X, op=OP.add)

            # box loss: sum oscl4 * (sel - aux4)^2 -> partials[0]
            d4 = tmp(4 * T)
            nc.vector.tensor_tensor(out=d4[:], in0=_v(selq[:], T, [[1, 4 * T]]),
                                    in1=_v(fp[:], O_AUX4, [[1, 4 * T]]),
                                    op=OP.subtract)
            d4m = tmp(4 * T)
            nc.vector.tensor_tensor(out=d4m[:], in0=d4[:],
                                    in1=_v(fp[:], O_OSCL4, [[1, 4 * T]]),
                                    op=OP.mult)
            ttr(d4m[:], d4[:], 0, 4 * T)

            # conf terms from u_sel: sum obj*u^2 -> [1], sum obj*u -> [2]
            OBJv = _v(fp[:], O_OBJ, [[1, T]])
            um = tmp(T)
            nc.vector.tensor_tensor(out=um[:], in0=_v(selq[:], 0, [[1, T]]),
                                    in1=OBJv, op=OP.mult)
            ttr(um[:], _v(selq[:], 0, [[1, T]]), 1, T)
            ttr(um[:], OBJv, 2, T)

            # cls: sum obj * ce_sel -> [3]
            ttr(_v(selq[:], 5 * T, [[1, T]]), OBJv, 3, T)

            if debug:
                dbg = pool.tile([P, 4 * TA], f32, name="dbg")
                nc.vector.tensor_copy(out=_v(dbg[:], 0, [[1, TA]]), in_=iou[:])
                nc.vector.tensor_copy(out=_v(dbg[:], TA, [[1, TA]]), in_=fmask[:])
                nc.vector.tensor_copy(out=_v(dbg[:], 2 * TA, [[1, TA]]), in_=u2[:])
                nc.vector.tensor_copy(out=_v(dbg[:], 3 * TA, [[1, TA]]), in_=inter[:])
                nc.sync.dma_start(out=dbg_d[:], in_=dbg[:])

            nc.sync.dma_start(out=partials_d[:], in_=partials[:])

    if split:
        _split_multi_waits(nc)
    return nc


# -------------------------------------------------------------- shard builder
def _make_in_maps(out, gt_boxes, anchor_np, gt_classes_np, num_box_np, T):
    obj, xo, yo, tw, th, cls_t = _build_target_np(gt_boxes, gt_classes_np,
                                                  num_box_np)
    SLOTS = P * T
    TA = T * A
    out_r = out.reshape(B, A, 25, HWC)
    sqa = np.sqrt(anchor_np)                       # [A, 2]

    in_maps = []
    for c in range(CORES):
        sl = slice(c * BC, (c + 1) * BC)
        ob = obj[sl]                               # [BC, HWC]
        bloc, hwloc = np.nonzero(ob > 0)
        K = len(bloc)
        assert K <= SLOTS

        def place(vals):
            buf = np.zeros(SLOTS, dtype=np.float32)
            buf[:K] = vals
            return buf.reshape(P, T)

        objv = place(np.ones(K, dtype=np.float32))
        xov = place(xo[sl][bloc, hwloc])
        yov = place(yo[sl][bloc, hwloc])
        twv = place(tw[sl][bloc, hwloc])
        thv = place(th[sl][bloc, hwloc])

        # occupied-cell prediction channels [K, A, 25] -> chans 20..24 f32,
        # logits 0..19 bf16
        colsb = np.zeros((SLOTS, A, 25), dtype=np.float32)
        if K:
            colsb[:K] = out_r[sl].transpose(0, 3, 1, 2)[bloc, hwloc]
        cols_xw = colsb[:, :, 20:25].reshape(P, T, A, 5)        # (t,a,{c,x,y,w,h})
        # fpack wants (t, 5ch, a): chan-major within cell
        cols_xw = np.ascontiguousarray(
            cols_xw.transpose(0, 1, 3, 2)).reshape(P, 25 * T)
        logits = np.ascontiguousarray(
            colsb[:, :, :20]).reshape(P, 100 * T)               # (t,a,j)

        # target-class logit per (t, a)
        clsv = place(cls_t[sl][bloc, hwloc].astype(np.float32)).astype(np.int64)
        s_aux = np.take_along_axis(
            colsb[:, :, :20].reshape(SLOTS, A, 20),
            clsv.reshape(SLOTS, 1, 1).repeat(A, axis=1), axis=2
        )[:, :, 0].reshape(P, TA).astype(np.float32)

        # xi-space target box edges (t, {x,y}): center 2o-1, half-width t_wh
        cxv = 2.0 * xov - 1.0
        cyv = 2.0 * yov - 1.0
        b1 = np.stack([cxv - twv, cyv - thv], axis=1).reshape(P, 2 * T)
        b2 = np.stack([cxv + twv, cyv + thv], axis=1).reshape(P, 2 * T)
        tarea = (twv * thv).reshape(P, T)

        wconst = np.broadcast_to(A - np.arange(A, dtype=np.float32), (P, A))
        sqav = np.broadcast_to(sqa.reshape(1, 10), (P, 10))

        aux4 = np.stack([cxv, cyv, np.sqrt(twv), np.sqrt(thv)],
                        axis=1).reshape(P, 4 * T)
        oscl4 = np.stack([0.25 * objv, 0.25 * objv, objv, objv],
                         axis=1).reshape(P, 4 * T)

        fpack = np.concatenate(
            [cols_xw, b1, b2, tarea, wconst, sqav,
             s_aux, aux4, oscl4, objv.reshape(P, T)], axis=1)

        # dense conf channels: [BC, A, HWC] -> [P, 1280] bf16
        confd = out_r[sl][:, :, 20, :].reshape(P, -1)

        in_maps.append({
            "fpack": np.ascontiguousarray(fpack, dtype=np.float32),
            "lgpack": _bf16(logits),
            "confd": _bf16(confd),
        })
    return in_maps


def _combine(results, ks):
    box_s = confu2 = confu1 = cls_s = 0.0
    du2 = du1 = 0.0
    for c in range(CORES):
        pr = results[c]["partials"].astype(np.float64)
        box_s += pr[:, 0].sum()
        confu2 += pr[:, 1].sum()
        confu1 += pr[:, 2].sum()
        cls_s += pr[:, 3].sum()
        du2 += pr[:, 4].sum()
        du1 += pr[:, 5].sum()
    dense = 0.25 * (float(B * A * HWC) + 2.0 * du1 + du2)
    K = float(sum(ks))
    # conf: 0.25*sum obj*(u-1)^2 ; noobj corr: 0.25*sum obj*(u+1)^2
    conf_s = 0.25 * (confu2 - 2.0 * confu1 + K)
    nob_c = 0.25 * (confu2 + 2.0 * confu1 + K)
    box_loss = np.float32(LAM_COORD / B * box_s)
    conf_loss = np.float32(LAM_OBJ / B * conf_s)
    noobj_loss = np.float32(LAM_NOOBJ / B * (dense - nob_c))
    cls_loss = np.float32(LAM_CLS / B * cls_s)
    return (box_loss, conf_loss, noobj_loss, cls_loss)


# ---------------------------------------------------------------- entry point
def kernel(out, gt_boxes, anchor, gt_classes, num_box):
    from concourse.bass_utils import run_bass_kernel_spmd

    out = np.ascontiguousarray(np.asarray(out, dtype=np.float32))
    gt_boxes = np.asarray(gt_boxes, dtype=np.float32)
    anchor_np = np.asarray(anchor, dtype=np.float32)
    gt_classes_np = np.asarray(gt_classes)
    num_box_np = np.asarray(num_box)

    # per-core occupied-cell counts decide the compiled tile factor T
    obj = _build_target_np(gt_boxes, gt_classes_np, num_box_np)[0]
    ks = [int((obj[c * BC:(c + 1) * BC] > 0).sum()) for c in range(CORES)]
    maxk = max(ks)
    T = 9 if maxk <= 9 * P else 13
    assert maxk <= 13 * P

    in_maps = _make_in_maps(out, gt_boxes, anchor_np, gt_classes_np,
                            num_box_np, T)

    import os
    key = f"nc{T}"
    if key not in _CACHE:
        _CACHE[key] = _build_nc(T)
    trace = os.environ.get("KERNEL_TRACE", "0") == "1"
    res = run_bass_kernel_spmd(_CACHE[key], in_maps, core_ids=list(range(CORES)),
                               trace=trace)
    if trace:
        print(f"HW exec time: {res.exec_time_ns} ns  (mean {res.mean_exec_time_ns})")
    return _combine(res.results, ks)


# revision 20
# speedup vs baseline: 1.0265x; 1.0265x over previous
"""Trainium2 Bass kernel for nn_Loss_65781719105930 (YOLO-style detection loss).

Strategy (pure data parallelism, 8 cores, 32 images each):
  host:   replicate the reference's target-build scatter (small int64 inputs),
          compact occupied cells, pre-pack aux tables + prediction columns into
          three contiguous DMA payloads; gather the target-class logit per
          (cell, anchor) host-side.
  device: dense pass over the 5 conf channels (sum of sigmoid^2), plus IoU /
          first-argmax / best-anchor-select / cross-entropy on compacted tiles.

Numeric tricks that keep the scalar engine on ONE activation-table set
(exp_and_others = {tanh, exp, square}):
  sigmoid(x)   = (1 + tanh(x/2)) / 2      -> work in xi = 2x-1 coords; the
                                             0.5 factors fold into host consts
  sqrt(exp(x)*anchor) = exp(x/2)*sqrt(anchor)
  ln(x)        ~ bitcast_i16(bf16 x) * ln2/2^7 - 126.94269504*ln2
                 (mean-centered log2 bit trick; loss tolerance is 2e-2 rel)

The grid offset cancels algebraically in both the IoU and the box loss, so it
never appears on device.

Device program layout: SRC [P, 7*T*A] holds quantities q = (w, h, x, y, u, uu,
ce) per (cell t, anchor a); one mul by (fmask*obj) + one reduce selects the
best anchor for all seven; one final grouped reduce produces all loss partial
sums at once.
"""
import numpy as np

# ---------------------------------------------------------------- constants
NCLS = 20
H = W = 32
HWC = H * W            # 1024 cells/image
A = 5
M = 50
B = 256
CORES = 8
BC = B // CORES        # 32 images per core
CH = A * (5 + NCLS)    # 125 channels
P = 128
LAM_COORD, LAM_OBJ, LAM_NOOBJ, LAM_CLS = 5.0, 1.0, 0.5, 1.0

LN2 = float(np.log(2.0))
LOG_BIAS = 126.94269504   # mean-centering constant for the log2 bit trick

_CACHE = {}


def _bf16(x):
    """float32 ndarray -> ml_dtypes.bfloat16 (RNE)."""
    import ml_dtypes
    return np.asarray(x, dtype=np.float32).astype(ml_dtypes.bfloat16)


# ---------------------------------------------------------------- host prep
def _build_target_np(gt_boxes, gt_classes, num_box):
    """Numpy replication of reference.build_target (last object wins, first-max
    class argmax). Returns per-cell [B, HWC] arrays."""
    Bn = gt_boxes.shape[0]
    valid = np.arange(M)[None, :] < num_box[:, None]
    x = gt_boxes[..., 0].astype(np.float32) * H
    y = gt_boxes[..., 1].astype(np.float32) * H
    gx = np.floor(x).astype(np.int64)
    gy = np.floor(y).astype(np.int64)
    flat = np.where(valid, gy * W + gx, HWC)
    bi = np.broadcast_to(np.arange(Bn)[:, None], (Bn, M))

    vals = np.stack([np.ones_like(x), x - gx, y - gy,
                     gt_boxes[..., 2].astype(np.float32) * H,
                     gt_boxes[..., 3].astype(np.float32) * H], axis=-1)
    tgt_box = np.zeros((Bn, HWC + 1, 5), dtype=np.float32)
    tgt_box[bi, flat] = vals
    tgt_cls = np.zeros((Bn, HWC + 1, NCLS), dtype=np.float32)
    tgt_cls[bi, flat, gt_classes.astype(np.int64)] = 1.0

    tgt_box = tgt_box[:, :HWC]
    obj = tgt_box[..., 0]
    cls_t = np.argmax(tgt_cls[:, :HWC], axis=-1).astype(np.int64)
    return obj, tgt_box[..., 1], tgt_box[..., 2], tgt_box[..., 3], tgt_box[..., 4], cls_t


def _split_multi_waits(nc):
    """This container's walrus accepts only ONE sem-wait per instruction; hoist
    extra waits onto standalone NoOps."""
    import concourse.mybir as mybir
    import bass_rust
    n = 0
    for fn in nc.m.functions:
        for blk in fn.blocks:
            new = []
            for ins in blk.instructions:
                si = ins.sync_info
                waits = list(si.on_wait) if si is not None else []
                if len(waits) > 1:
                    for w in waits[:-1]:
                        nop = mybir.InstNoOp(name=f"{ins.name}-w{n}")
                        nop.engine = ins.engine
                        nop.sync_info = bass_rust.SyncInfo(on_wait=[w], on_update=[])
                        new.append(nop)
                        n += 1
                    si.on_wait = [waits[-1]]
                    ins.sync_info = si
                new.append(ins)
            blk.instructions = new
    return n


def _offsets(T):
    """fpack free-dim offsets. cols_xw channel order is (x, y, conf, w, h)."""
    o = {}
    o["XW"] = 0
    o["B1"] = 25 * T
    o["B2"] = o["B1"] + 2 * T
    o["TAREA"] = o["B2"] + 2 * T
    o["WCONST"] = o["TAREA"] + T
    o["SQA"] = o["WCONST"] + 5
    o["SAUX"] = o["SQA"] + 10
    o["AUX4"] = o["SAUX"] + 5 * T
    o["OSCL4"] = o["AUX4"] + 4 * T
    o["OBJ"] = o["OSCL4"] + 4 * T
    o["NF"] = o["OBJ"] + T
    return o


# ---------------------------------------------------------------- bass build
def _build_nc(T, split=True):
    """Build the per-core kernel for T cell-blocks per partition (P*T slots).

    fpack [P, NF] f32:
      cols_xw (t, ch{x,y,conf,w,h}, a)  25T
      B1, B2  (d{x,y}, t)               2T each   xi-space target box edges
      TAREA   (t)                       T         tw*th (physical, cell units)
      WCONST  (a)                       5         A - a (first-argmax tiebreak)
      SQA     (d{w,h}, a)               10        sqrt(anchor)
      S_AUX   (t, a)                    5T        target-class logit
      AUX4    (q{w,h,x,y}, t)           4T        (sqrt tw, sqrt th, 2xo-1, 2yo-1)
      OSCL4   (q, t)                    4T        obj * {1,1,.25,.25}
      OBJ     (t)                       T
    lgpack bf16 [P, 100T]: logits (t, a, j)
    confd  bf16 [P, 1280]: all conf channels (dense noobj pass)
    partials out [P, 12]:
      0..3 box (w,h,x,y) sq-diff sums, 4 sum obj*u, 5 sum obj*u^2,
      6 sum obj*ce, 7 dense sum ud, 8 dense sum ud^2   (u/ud = tanh(conf/2))
    """
    import concourse.bass as bass
    import concourse.mybir as mybir
    import concourse.tile as tile

    f32 = mybir.dt.float32
    bf16 = mybir.dt.bfloat16
    i16 = mybir.dt.int16
    AF = mybir.ActivationFunctionType
    OP = mybir.AluOpType
    AX = mybir.AxisListType

    TA = T * A
    TA2 = TA * 2
    O = _offsets(T)
    NF = O["NF"]
    DF = BC * A * HWC // P   # 1280 dense conf elements per partition

    def _v(ap, off, dims):
        """Sub-view of a tile AP: keep its partition dim, replace free dims."""
        return bass.AP(tensor=ap.tensor, offset=ap.offset + off,
                       ap=[list(ap.ap[0])] + dims)

    nc = bass.Bass("TRN2")
    fpack_d = nc.declare_dram_parameter("fpack", [P, NF], f32, isOutput=False)
    lgpack_d = nc.declare_dram_parameter("lgpack", [P, 100 * T], bf16, isOutput=False)
    confd_d = nc.declare_dram_parameter("confd", [P, DF], bf16, isOutput=False)
    partials_d = nc.declare_dram_parameter("partials", [P, 12], f32, isOutput=True)

    with tile.TileContext(nc) as tc:
        with tc.tile_pool(name="sb", bufs=1) as pool:
            # ---------------- input DMAs, priority order, on sync HWDGE
            fp = pool.tile([P, NF], f32, name="fp")
            nc.sync.dma_start(out=fp[:], in_=fpack_d[:])
            lg_in = pool.tile([P, 100 * T], bf16, name="lg_in")
            nc.sync.dma_start(out=lg_in[:], in_=lgpack_d[:])
            confd = pool.tile([P, DF], bf16, name="confd")
            nc.sync.dma_start(out=confd[:], in_=confd_d[:])

            partials = pool.tile([P, 12], f32, name="partials")

            # SRC: (q, t, a) with q in {w, h, x, y, u, uu, ce}
            SRC = pool.tile([P, 7 * TA], f32, name="SRC")

            # ---------------- scalar engine program (one act-table set)
            # EW = exp(chan{w,h}/2): chans 3,4 of cols_xw -> (t, a, d)
            EW = pool.tile([P, TA2], f32, name="EW")
            nc.scalar.activation(
                _v(EW[:], 0, [[1, 2], [2 * A, T], [2, A]]),
                _v(fp[:], O["XW"] + 15, [[5, 2], [25, T], [1, A]]),
                AF.Exp, scale=0.5)
            # x, y, u = tanh(chan{x,y,conf}/2) -> SRC q2, q3, q4
            nc.scalar.activation(
                _v(SRC[:], 2 * TA, [[TA, 3], [A, T], [1, A]]),
                _v(fp[:], O["XW"], [[5, 3], [25, T], [1, A]]),
                AF.Tanh, scale=0.5)
            # e = exp(logits), bf16, (t, a, j)
            e = pool.tile([P, 100 * T], bf16, name="e")
            nc.scalar.activation(
                _v(e[:], 0, [[100, T], [NCLS, A], [1, NCLS]]),
                _v(lg_in[:], 0, [[100, T], [NCLS, A], [1, NCLS]]),
                AF.Exp)
            # dense noobj pass: ud = tanh(c/2); sum sigma^2 = 0.25*(N + 2*sum ud
            # + sum ud^2)
            UD = pool.tile([P, DF], f32, name="UD")
            nc.scalar.activation(UD[:], confd[:], AF.Tanh, scale=0.5,
                                 accum_out=_v(partials[:], 7, [[1, 1]]))
            SQD = pool.tile([P, DF], f32, name="SQD")
            nc.scalar.activation(SQD[:], UD[:], AF.Square,
                                 accum_out=_v(partials[:], 8, [[1, 1]]))

            # ---------------- vector engine program
            tcnt = [0]

            def tmp(n, dtype=f32):
                tcnt[0] += 1
                return pool.tile([P, n], dtype, name=f"t{tcnt[0]}")

            # sh = EW * sqrt(anchor) = sqrt(pred_wh) -> SRC q0, q1
            SH = _v(SRC[:], 0, [[TA, 2], [A, T], [1, A]])
            nc.vector.tensor_tensor(
                out=SH,
                in0=_v(EW[:], 0, [[1, 2], [2 * A, T], [2, A]]),
                in1=_v(fp[:], O["SQA"], [[1, 2], [0, T], [2, A]]),
                op=OP.mult)
            # wfull = sh*sh = pred_wh (xi-space half-width), (d, t, a)
            wf = tmp(TA2)
            SH2 = _v(SRC[:], 0, [[TA, 2], [1, TA]])
            WF = _v(wf[:], 0, [[TA, 2], [1, TA]])
            nc.vector.tensor_tensor(out=WF, in0=SH2, in1=SH2, op=OP.mult)

            # IoU in xi coords. XY = SRC q2, q3 as (d, t, a)
            XY = _v(SRC[:], 2 * TA, [[TA, 2], [1, TA]])
            lo = tmp(TA2)
            nc.vector.tensor_tensor(out=lo[:], in0=XY, in1=WF, op=OP.subtract)
            hi = tmp(TA2)
            nc.vector.tensor_tensor(out=hi[:], in0=XY, in1=WF, op=OP.add)
            B1v = _v(fp[:], O["B1"], [[T, 2], [1, T], [0, A]])
            B2v = _v(fp[:], O["B2"], [[T, 2], [1, T], [0, A]])
            LOv = _v(lo[:], 0, [[TA, 2], [A, T], [1, A]])
            HIv = _v(hi[:], 0, [[TA, 2], [A, T], [1, A]])
            t1 = tmp(TA2)
            nc.vector.tensor_tensor(out=_v(t1[:], 0, [[TA, 2], [A, T], [1, A]]),
                                    in0=HIv, in1=B2v, op=OP.min)
            t2 = tmp(TA2)
            nc.vector.tensor_tensor(out=_v(t2[:], 0, [[TA, 2], [A, T], [1, A]]),
                                    in0=LOv, in1=B1v, op=OP.max)
            t3 = tmp(TA2)
            nc.vector.tensor_tensor(out=t3[:], in0=t1[:], in1=t2[:], op=OP.subtract)
            # iw = max(t3, 0) * 0.5 -> physical overlap widths (d, t, a)
            iwih = tmp(TA2)
            nc.vector.tensor_scalar(out=iwih[:], in0=t3[:], scalar1=0.0,
                                    scalar2=0.5, op0=OP.max, op1=OP.mult)
            inter = tmp(TA)
            nc.vector.tensor_tensor(out=inter[:], in0=_v(iwih[:], 0, [[1, TA]]),
                                    in1=_v(iwih[:], TA, [[1, TA]]), op=OP.mult)
            areaA = tmp(TA)
            nc.vector.tensor_tensor(out=areaA[:], in0=_v(wf[:], 0, [[1, TA]]),
                                    in1=_v(wf[:], TA, [[1, TA]]), op=OP.mult)
            u1 = tmp(TA)
            nc.vector.tensor_tensor(out=_v(u1[:], 0, [[A, T], [1, A]]),
                                    in0=_v(areaA[:], 0, [[A, T], [1, A]]),
                                    in1=_v(fp[:], O["TAREA"], [[1, T], [0, A]]),
                                    op=OP.add)
            u2 = tmp(TA)
            nc.vector.tensor_tensor(out=u2[:], in0=u1[:], in1=inter[:],
                                    op=OP.subtract)
            rcp = tmp(TA)
            nc.vector.reciprocal(out=rcp[:], in_=u2[:])
            iou = tmp(TA)
            nc.vector.tensor_tensor(out=iou[:], in0=inter[:], in1=rcp[:],
                                    op=OP.mult)

            # first-argmax over a -> fmask (exact float equality + tiebreak)
            rmax = tmp(T)
            nc.vector.tensor_reduce(out=rmax[:],
                                    in_=_v(iou[:], 0, [[A, T], [1, A]]),
                                    axis=AX.X, op=OP.max)
            eq = tmp(TA)
            nc.vector.tensor_tensor(out=_v(eq[:], 0, [[A, T], [1, A]]),
                                    in0=_v(iou[:], 0, [[A, T], [1, A]]),
                                    in1=_v(rmax[:], 0, [[1, T], [0, A]]),
                                    op=OP.is_equal)
            fval = tmp(TA)
            nc.vector.tensor_tensor(out=_v(fval[:], 0, [[A, T], [1, A]]),
                                    in0=_v(eq[:], 0, [[A, T], [1, A]]),
                                    in1=_v(fp[:], O["WCONST"], [[0, T], [1, A]]),
                                    op=OP.mult)
            m2 = tmp(T)
            nc.vector.tensor_reduce(out=m2[:],
                                    in_=_v(fval[:], 0, [[A, T], [1, A]]),
                                    axis=AX.X, op=OP.max)
            fmask = tmp(TA)
            nc.vector.tensor_tensor(out=_v(fmask[:], 0, [[A, T], [1, A]]),
                                    in0=_v(fval[:], 0, [[A, T], [1, A]]),
                                    in1=_v(m2[:], 0, [[1, T], [0, A]]),
                                    op=OP.is_equal)
            # fobj = fmask * obj (premasks every selected quantity)
            fobj = tmp(TA)
            nc.vector.tensor_tensor(out=_v(fobj[:], 0, [[A, T], [1, A]]),
                                    in0=_v(fmask[:], 0, [[A, T], [1, A]]),
                                    in1=_v(fp[:], O["OBJ"], [[1, T], [0, A]]),
                                    op=OP.mult)
            # uu = u^2 -> SRC q5
            nc.vector.tensor_tensor(out=_v(SRC[:], 5 * TA, [[1, TA]]),
                                    in0=_v(SRC[:], 4 * TA, [[1, TA]]),
                                    in1=_v(SRC[:], 4 * TA, [[1, TA]]),
                                    op=OP.mult)

            # cls path: se = sum_j e (bf16), lse via bf16 bit trick (gpsimd),
            # ce = lse - s -> SRC q6
            se = tmp(TA, bf16)
            with nc.allow_low_precision(reason="lse bit-trick needs bf16 bits; "
                                        "2e-2 loss tolerance"):
                nc.vector.tensor_reduce(
                    out=_v(se[:], 0, [[A, T], [1, A]]),
                    in_=_v(e[:], 0, [[100, T], [NCLS, A], [1, NCLS]]),
                    axis=AX.X, op=OP.add)
            lgf = tmp(TA)
            nc.gpsimd.tensor_copy(out=lgf[:], in_=se[:].bitcast(i16))
            lse = tmp(TA)
            nc.gpsimd.tensor_scalar(out=lse[:], in0=lgf[:],
                                    scalar1=LN2 / (1 << 7),
                                    scalar2=-LOG_BIAS * LN2,
                                    op0=OP.mult, op1=OP.add)
            nc.gpsimd.tensor_tensor(out=_v(SRC[:], 6 * TA, [[A, T], [1, A]]),
                                    in0=_v(lse[:], 0, [[A, T], [1, A]]),
                                    in1=_v(fp[:], O["SAUX"], [[A, T], [1, A]]),
                                    op=OP.subtract)

            # best-anchor selection of all seven quantities in one mul+reduce
            selm = pool.tile([P, 7 * TA], f32, name="selm")
            nc.vector.tensor_tensor(out=_v(selm[:], 0, [[TA, 7], [1, TA]]),
                                    in0=_v(SRC[:], 0, [[TA, 7], [1, TA]]),
                                    in1=_v(fobj[:], 0, [[0, 7], [1, TA]]),
                                    op=OP.mult)
            FIN = pool.tile([P, 7 * T], f32, name="FIN")
            nc.vector.tensor_reduce(out=_v(FIN[:], 0, [[T, 7], [1, T]]),
                                    in_=_v(selm[:], 0, [[TA, 7], [A, T], [1, A]]),
                                    axis=AX.X, op=OP.add)

            # box: FIN q0..3 -> oscl4*(sel - aux4)^2, written back into q0..3
            d4 = tmp(4 * T)
            nc.vector.tensor_tensor(out=d4[:], in0=_v(FIN[:], 0, [[1, 4 * T]]),
                                    in1=_v(fp[:], O["AUX4"], [[1, 4 * T]]),
                                    op=OP.subtract)
            d4m = tmp(4 * T)
            nc.vector.tensor_tensor(out=d4m[:], in0=d4[:],
                                    in1=_v(fp[:], O["OSCL4"], [[1, 4 * T]]),
                                    op=OP.mult)
            nc.vector.tensor_tensor(out=_v(FIN[:], 0, [[1, 4 * T]]),
                                    in0=d4[:], in1=d4m[:], op=OP.mult)

            # one grouped reduce -> partials cols 0..6
            nc.vector.tensor_reduce(out=_v(partials[:], 0, [[1, 7]]),
                                    in_=_v(FIN[:], 0, [[T, 7], [1, T]]),
                                    axis=AX.X, op=OP.add)

            nc.sync.dma_start(out=partials_d[:], in_=partials[:])

    if split:
        _split_multi_waits(nc)
    return nc


# -------------------------------------------------------------- shard builder
def _make_in_maps(out, gt_boxes, anchor_np, gt_classes_np, num_box_np, T):
    obj, xo, yo, tw, th, cls_t = _build_target_np(gt_boxes, gt_classes_np,
                                                  num_box_np)
    SLOTS = P * T
    TA = T * A
    out_r = out.reshape(B, A, 25, HWC)
    sqa = np.sqrt(anchor_np)                       # [A, 2]

    in_maps = []
    for c in range(CORES):
        sl = slice(c * BC, (c + 1) * BC)
        ob = obj[sl]                               # [BC, HWC]
        bloc, hwloc = np.nonzero(ob > 0)
        K = len(bloc)
        assert K <= SLOTS

        def place(vals):
            buf = np.zeros(SLOTS, dtype=np.float32)
            buf[:K] = vals
            return buf.reshape(P, T)

        objv = place(np.ones(K, dtype=np.float32))
        xov = place(xo[sl][bloc, hwloc])
        yov = place(yo[sl][bloc, hwloc])
        twv = place(tw[sl][bloc, hwloc])
        thv = place(th[sl][bloc, hwloc])

        # occupied-cell prediction channels [K, A, 25]
        colsb = np.zeros((SLOTS, A, 25), dtype=np.float32)
        if K:
            colsb[:K] = out_r[sl].transpose(0, 3, 1, 2)[bloc, hwloc]
        # cols_xw (t, ch{x,y,conf,w,h}, a)
        sel = colsb[:, :, [21, 22, 20, 23, 24]]            # (t, a, ch)
        cols_xw = np.ascontiguousarray(
            sel.reshape(P, T, A, 5).transpose(0, 1, 3, 2)).reshape(P, 25 * T)
        logits = np.ascontiguousarray(
            colsb[:, :, :20]).reshape(P, 100 * T)          # (t, a, j)

        # target-class logit per (t, a)
        clsv = place(cls_t[sl][bloc, hwloc].astype(np.float32)).astype(np.int64)
        s_aux = np.take_along_axis(
            colsb[:, :, :20].reshape(SLOTS, A, 20),
            clsv.reshape(SLOTS, 1, 1).repeat(A, axis=1), axis=2
        )[:, :, 0].reshape(P, TA).astype(np.float32)

        # xi-space target box edges (d{x,y}, t): center 2o-1, half-width t_wh
        cxv = 2.0 * xov - 1.0
        cyv = 2.0 * yov - 1.0
        b1 = np.stack([cxv - twv, cyv - thv], axis=1).reshape(P, 2 * T)
        b2 = np.stack([cxv + twv, cyv + thv], axis=1).reshape(P, 2 * T)
        tarea = (twv * thv).reshape(P, T)

        wconst = np.broadcast_to(A - np.arange(A, dtype=np.float32), (P, A))
        sqav = np.broadcast_to(sqa.reshape(1, 10), (P, 10))

        # AUX4 / OSCL4 in q-order (w, h, x, y)
        aux4 = np.stack([np.sqrt(twv), np.sqrt(thv), cxv, cyv],
                        axis=1).reshape(P, 4 * T)
        oscl4 = np.stack([objv, objv, 0.25 * objv, 0.25 * objv],
                         axis=1).reshape(P, 4 * T)

        fpack = np.concatenate(
            [cols_xw, b1, b2, tarea, wconst, sqav,
             s_aux, aux4, oscl4, objv.reshape(P, T)], axis=1)

        # dense conf channels: [BC, A, HWC] -> [P, 1280] bf16
        confd = out_r[sl][:, :, 20, :].reshape(P, -1)

        in_maps.append({
            "fpack": np.ascontiguousarray(fpack, dtype=np.float32),
            "lgpack": _bf16(logits),
            "confd": _bf16(confd),
        })
    return in_maps


# ---------------------------------------------------------------- entry point
def kernel(out, gt_boxes, anchor, gt_classes, num_box):
    from concourse.bass_utils import run_bass_kernel_spmd

    out = np.ascontiguousarray(np.asarray(out, dtype=np.float32))
    gt_boxes = np.asarray(gt_boxes, dtype=np.float32)
    anchor_np = np.asarray(anchor, dtype=np.float32)
    gt_classes_np = np.asarray(gt_classes)
    num_box_np = np.asarray(num_box)

    # per-core occupied-cell counts decide the compiled tile factor T
    obj = _build_target_np(gt_boxes, gt_classes_np, num_box_np)[0]
    ks = [int((obj[c * BC:(c + 1) * BC] > 0).sum()) for c in range(CORES)]
    maxk = max(ks)
    T = max(1, -(-maxk // P))
    assert maxk <= 13 * P and T <= 13

    in_maps = _make_in_maps(out, gt_boxes, anchor_np, gt_classes_np,
                            num_box_np, T)

    import os
    key = f"nc{T}"
    if key not in _CACHE:
        _CACHE[key] = _build_nc(T)
    trace = os.environ.get("KERNEL_TRACE", "0") == "1"
    res = run_bass_kernel_spmd(_CACHE[key], in_maps, core_ids=list(range(CORES)),
                               trace=trace)
    if trace:
        print(f"HW exec time: {res.exec_time_ns} ns  (mean {res.mean_exec_time_ns})")

    cols = np.zeros(12, dtype=np.float64)
    for c in range(CORES):
        cols += res.results[c]["partials"].astype(np.float64).sum(axis=0)
    K = float(sum(ks))
    box_loss = np.float32(LAM_COORD / B * (cols[0] + cols[1] + cols[2] + cols[3]))
    conf_loss = np.float32(LAM_OBJ / B * 0.25 * (cols[5] - 2.0 * cols[4] + K))
    nob_c = 0.25 * (cols[5] + 2.0 * cols[4] + K)
    dense = 0.25 * (float(B * A * HWC) + 2.0 * cols[7] + cols[8])
    noobj_loss = np.float32(LAM_NOOBJ / B * (dense - nob_c))
    cls_loss = np.float32(LAM_CLS / B * cols[6])
    return (box_loss, conf_loss, noobj_loss, cls_loss)


# revision 22
# speedup vs baseline: 1.0367x; 1.0100x over previous
"""Trainium2 Bass kernel for nn_Loss_65781719105930 (YOLO-style detection loss).

Strategy (pure data parallelism, 8 cores, 32 images each):
  host:   replicate the reference's target-build scatter (small int64 inputs),
          compact occupied cells, pre-pack aux tables + prediction columns into
          three contiguous DMA payloads; gather the target-class logit per
          (cell, anchor) host-side.
  device: dense pass over the 5 conf channels (sum of sigmoid^2), plus IoU /
          first-argmax / best-anchor-select / cross-entropy on compacted tiles.

Numeric tricks that keep the scalar engine on ONE activation-table set
(exp_and_others = {tanh, exp, square}):
  sigmoid(x)   = (1 + tanh(x/2)) / 2      -> work in xi = 2x-1 coords; the
                                             0.5 factors fold into host consts
  sqrt(exp(x)*anchor) = exp(x/2)*sqrt(anchor)
  ln(x)        ~ bitcast_i16(bf16 x) * ln2/2^7 - 126.94269504*ln2
                 (mean-centered log2 bit trick; loss tolerance is 2e-2 rel)

The grid offset cancels algebraically in both the IoU and the box loss, so it
never appears on device.

Device program layout: SRC [P, 7*T*A] holds quantities q = (w, h, x, y, u, uu,
ce) per (cell t, anchor a); one mul by (fmask*obj) + one reduce selects the
best anchor for all seven; one final grouped reduce produces all loss partial
sums at once.
"""
import numpy as np

# ---------------------------------------------------------------- constants
NCLS = 20
H = W = 32
HWC = H * W            # 1024 cells/image
A = 5
M = 50
B = 256
CORES = 8
BC = B // CORES        # 32 images per core
CH = A * (5 + NCLS)    # 125 channels
P = 128
LAM_COORD, LAM_OBJ, LAM_NOOBJ, LAM_CLS = 5.0, 1.0, 0.5, 1.0

LN2 = float(np.log(2.0))
LOG_BIAS = 126.94269504   # mean-centering constant for the log2 bit trick

_CACHE = {}


def _bf16(x):
    """float32 ndarray -> ml_dtypes.bfloat16 (RNE)."""
    import ml_dtypes
    return np.asarray(x, dtype=np.float32).astype(ml_dtypes.bfloat16)


# ---------------------------------------------------------------- host prep
def _build_target_np(gt_boxes, gt_classes, num_box):
    """Numpy replication of reference.build_target (last object wins, first-max
    class argmax). Returns per-cell [B, HWC] arrays."""
    Bn = gt_boxes.shape[0]
    valid = np.arange(M)[None, :] < num_box[:, None]
    x = gt_boxes[..., 0].astype(np.float32) * H
    y = gt_boxes[..., 1].astype(np.float32) * H
    gx = np.floor(x).astype(np.int64)
    gy = np.floor(y).astype(np.int64)
    flat = np.where(valid, gy * W + gx, HWC)
    bi = np.broadcast_to(np.arange(Bn)[:, None], (Bn, M))

    vals = np.stack([np.ones_like(x), x - gx, y - gy,
                     gt_boxes[..., 2].astype(np.float32) * H,
                     gt_boxes[..., 3].astype(np.float32) * H], axis=-1)
    tgt_box = np.zeros((Bn, HWC + 1, 5), dtype=np.float32)
    tgt_box[bi, flat] = vals
    tgt_cls = np.zeros((Bn, HWC + 1, NCLS), dtype=np.float32)
    tgt_cls[bi, flat, gt_classes.astype(np.int64)] = 1.0

    tgt_box = tgt_box[:, :HWC]
    obj = tgt_box[..., 0]
    cls_t = np.argmax(tgt_cls[:, :HWC], axis=-1).astype(np.int64)
    return obj, tgt_box[..., 1], tgt_box[..., 2], tgt_box[..., 3], tgt_box[..., 4], cls_t


def _split_multi_waits(nc):
    """This container's walrus accepts only ONE sem-wait per instruction; hoist
    extra waits onto standalone NoOps."""
    import concourse.mybir as mybir
    import bass_rust
    n = 0
    for fn in nc.m.functions:
        for blk in fn.blocks:
            new = []
            for ins in blk.instructions:
                si = ins.sync_info
                waits = list(si.on_wait) if si is not None else []
                if len(waits) > 1:
                    for w in waits[:-1]:
                        nop = mybir.InstNoOp(name=f"{ins.name}-w{n}")
                        nop.engine = ins.engine
                        nop.sync_info = bass_rust.SyncInfo(on_wait=[w], on_update=[])
                        new.append(nop)
                        n += 1
                    si.on_wait = [waits[-1]]
                    ins.sync_info = si
                new.append(ins)
            blk.instructions = new
    return n


def _offsets(T):
    """fpack free-dim offsets. cols_xw channel order is (x, y, conf, w, h)."""
    o = {}
    o["XW"] = 0
    o["B1"] = 25 * T
    o["B2"] = o["B1"] + 2 * T
    o["TAREA"] = o["B2"] + 2 * T
    o["WCONST"] = o["TAREA"] + T
    o["SQA"] = o["WCONST"] + 5
    o["SAUX"] = o["SQA"] + 10
    o["AUX4"] = o["SAUX"] + 5 * T
    o["OSCL4"] = o["AUX4"] + 4 * T
    o["OBJ"] = o["OSCL4"] + 4 * T
    o["NF"] = o["OBJ"] + T
    return o


# ---------------------------------------------------------------- bass build
def _build_nc(T, split=True):
    """Build the per-core kernel for T cell-blocks per partition (P*T slots).

    fpack [P, NF] f32:
      cols_xw (t, ch{x,y,conf,w,h}, a)  25T
      B1, B2  (d{x,y}, t)               2T each   xi-space target box edges
      TAREA   (t)                       T         tw*th (physical, cell units)
      WCONST  (a)                       5         A - a (first-argmax tiebreak)
      SQA     (d{w,h}, a)               10        sqrt(anchor)
      S_AUX   (t, a)                    5T        target-class logit
      AUX4    (q{w,h,x,y}, t)           4T        (sqrt tw, sqrt th, 2xo-1, 2yo-1)
      OSCL4   (q, t)                    4T        obj * {1,1,.25,.25}
      OBJ     (t)                       T
    lgpack bf16 [P, 100T]: logits (t, a, j)
    confd  bf16 [P, 1280]: all conf channels (dense noobj pass)
    partials out [P, 12]:
      0..3 box (w,h,x,y) sq-diff sums, 4 sum obj*u, 5 sum obj*u^2,
      6 sum obj*ce, 7 dense sum ud, 8 dense sum ud^2   (u/ud = tanh(conf/2))
    """
    import concourse.bass as bass
    import concourse.mybir as mybir
    import concourse.tile as tile

    f32 = mybir.dt.float32
    bf16 = mybir.dt.bfloat16
    i16 = mybir.dt.int16
    AF = mybir.ActivationFunctionType
    OP = mybir.AluOpType
    AX = mybir.AxisListType

    TA = T * A
    TA2 = TA * 2
    O = _offsets(T)
    NF = O["NF"]
    DF = BC * A * HWC // P   # 1280 dense conf elements per partition

    def _v(ap, off, dims):
        """Sub-view of a tile AP: keep its partition dim, replace free dims."""
        return bass.AP(tensor=ap.tensor, offset=ap.offset + off,
                       ap=[list(ap.ap[0])] + dims)

    nc = bass.Bass("TRN2")
    fpack_d = nc.declare_dram_parameter("fpack", [P, NF], f32, isOutput=False)
    lgpack_d = nc.declare_dram_parameter("lgpack", [P, 100 * T], bf16, isOutput=False)
    confd_d = nc.declare_dram_parameter("confd", [P, DF], bf16, isOutput=False)
    partials_d = nc.declare_dram_parameter("partials", [P, 12], f32, isOutput=True)

    with tile.TileContext(nc) as tc:
        with tc.tile_pool(name="sb", bufs=1) as pool:
            # ---------------- input DMAs, priority order, on sync HWDGE
            fp = pool.tile([P, NF], f32, name="fp")
            nc.sync.dma_start(out=fp[:], in_=fpack_d[:])
            lg_in = pool.tile([P, 100 * T], bf16, name="lg_in")
            nc.sync.dma_start(out=lg_in[:], in_=lgpack_d[:])
            confd = pool.tile([P, DF], bf16, name="confd")
            nc.sync.dma_start(out=confd[:], in_=confd_d[:])

            partials = pool.tile([P, 12], f32, name="partials")

            # SRC: (q, t, a) with q in {w, h, x, y, u, uu, ce}
            SRC = pool.tile([P, 7 * TA], f32, name="SRC")

            # ---------------- scalar engine program (one act-table set)
            # EW = exp(chan{w,h}/2): chans 3,4 of cols_xw -> (t, a, d)
            EW = pool.tile([P, TA2], f32, name="EW")
            nc.scalar.activation(
                _v(EW[:], 0, [[1, 2], [2 * A, T], [2, A]]),
                _v(fp[:], O["XW"] + 15, [[5, 2], [25, T], [1, A]]),
                AF.Exp, scale=0.5)
            # x, y, u = tanh(chan{x,y,conf}/2) -> SRC q2, q3, q4
            nc.scalar.activation(
                _v(SRC[:], 2 * TA, [[TA, 3], [A, T], [1, A]]),
                _v(fp[:], O["XW"], [[5, 3], [25, T], [1, A]]),
                AF.Tanh, scale=0.5)
            # e = exp(logits), bf16, (t, a, j)
            e = pool.tile([P, 100 * T], bf16, name="e")
            nc.scalar.activation(
                _v(e[:], 0, [[100, T], [NCLS, A], [1, NCLS]]),
                _v(lg_in[:], 0, [[100, T], [NCLS, A], [1, NCLS]]),
                AF.Exp)
            # dense noobj pass: ud = tanh(c/2); sum sigma^2 = 0.25*(N + 2*sum ud
            # + sum ud^2)
            # uu = u^2 -> SRC q5 (square is in the same act-table set)
            nc.scalar.activation(_v(SRC[:], 5 * TA, [[1, TA]]),
                                 _v(SRC[:], 4 * TA, [[1, TA]]), AF.Square)
            UD = pool.tile([P, DF], f32, name="UD")
            nc.scalar.activation(UD[:], confd[:], AF.Tanh, scale=0.5,
                                 accum_out=_v(partials[:], 7, [[1, 1]]))
            SQD = pool.tile([P, DF], f32, name="SQD")
            nc.scalar.activation(SQD[:], UD[:], AF.Square,
                                 accum_out=_v(partials[:], 8, [[1, 1]]))

            # ---------------- vector engine program
            tcnt = [0]

            def tmp(n, dtype=f32):
                tcnt[0] += 1
                return pool.tile([P, n], dtype, name=f"t{tcnt[0]}")

            # sh = EW * sqrt(anchor) = sqrt(pred_wh) -> SRC q0, q1
            SH = _v(SRC[:], 0, [[TA, 2], [A, T], [1, A]])
            nc.vector.tensor_tensor(
                out=SH,
                in0=_v(EW[:], 0, [[1, 2], [2 * A, T], [2, A]]),
                in1=_v(fp[:], O["SQA"], [[1, 2], [0, T], [2, A]]),
                op=OP.mult)
            # wfull = sh*sh = pred_wh (xi-space half-width), (d, t, a)
            wf = tmp(TA2)
            SH2 = _v(SRC[:], 0, [[TA, 2], [1, TA]])
            WF = _v(wf[:], 0, [[TA, 2], [1, TA]])
            nc.vector.tensor_tensor(out=WF, in0=SH2, in1=SH2, op=OP.mult)

            # IoU in xi coords. XY = SRC q2, q3 as (d, t, a)
            XY = _v(SRC[:], 2 * TA, [[TA, 2], [1, TA]])
            lo = tmp(TA2)
            nc.vector.tensor_tensor(out=lo[:], in0=XY, in1=WF, op=OP.subtract)
            hi = tmp(TA2)
            nc.vector.tensor_tensor(out=hi[:], in0=XY, in1=WF, op=OP.add)
            B1v = _v(fp[:], O["B1"], [[T, 2], [1, T], [0, A]])
            B2v = _v(fp[:], O["B2"], [[T, 2], [1, T], [0, A]])
            LOv = _v(lo[:], 0, [[TA, 2], [A, T], [1, A]])
            HIv = _v(hi[:], 0, [[TA, 2], [A, T], [1, A]])
            t1 = tmp(TA2)
            nc.vector.tensor_tensor(out=_v(t1[:], 0, [[TA, 2], [A, T], [1, A]]),
                                    in0=HIv, in1=B2v, op=OP.min)
            t2 = tmp(TA2)
            nc.vector.tensor_tensor(out=_v(t2[:], 0, [[TA, 2], [A, T], [1, A]]),
                                    in0=LOv, in1=B1v, op=OP.max)
            t3 = tmp(TA2)
            nc.vector.tensor_tensor(out=t3[:], in0=t1[:], in1=t2[:], op=OP.subtract)
            # iw = max(t3, 0) * 0.5 -> physical overlap widths (d, t, a)
            iwih = tmp(TA2)
            nc.vector.tensor_scalar(out=iwih[:], in0=t3[:], scalar1=0.0,
                                    scalar2=0.5, op0=OP.max, op1=OP.mult)
            inter = tmp(TA)
            nc.vector.tensor_tensor(out=inter[:], in0=_v(iwih[:], 0, [[1, TA]]),
                                    in1=_v(iwih[:], TA, [[1, TA]]), op=OP.mult)
            areaA = tmp(TA)
            nc.vector.tensor_tensor(out=areaA[:], in0=_v(wf[:], 0, [[1, TA]]),
                                    in1=_v(wf[:], TA, [[1, TA]]), op=OP.mult)
            u1 = tmp(TA)
            nc.vector.tensor_tensor(out=_v(u1[:], 0, [[A, T], [1, A]]),
                                    in0=_v(areaA[:], 0, [[A, T], [1, A]]),
                                    in1=_v(fp[:], O["TAREA"], [[1, T], [0, A]]),
                                    op=OP.add)
            u2 = tmp(TA)
            nc.vector.tensor_tensor(out=u2[:], in0=u1[:], in1=inter[:],
                                    op=OP.subtract)
            rcp = tmp(TA)
            nc.vector.reciprocal(out=rcp[:], in_=u2[:])
            iou = tmp(TA)
            nc.vector.tensor_tensor(out=iou[:], in0=inter[:], in1=rcp[:],
                                    op=OP.mult)

            # first-argmax over a -> fmask (exact float equality + tiebreak)
            rmax = tmp(T)
            nc.vector.tensor_reduce(out=rmax[:],
                                    in_=_v(iou[:], 0, [[A, T], [1, A]]),
                                    axis=AX.X, op=OP.max)
            eq = tmp(TA)
            nc.vector.tensor_tensor(out=_v(eq[:], 0, [[A, T], [1, A]]),
                                    in0=_v(iou[:], 0, [[A, T], [1, A]]),
                                    in1=_v(rmax[:], 0, [[1, T], [0, A]]),
                                    op=OP.is_equal)
            fval = tmp(TA)
            nc.vector.tensor_tensor(out=_v(fval[:], 0, [[A, T], [1, A]]),
                                    in0=_v(eq[:], 0, [[A, T], [1, A]]),
                                    in1=_v(fp[:], O["WCONST"], [[0, T], [1, A]]),
                                    op=OP.mult)
            m2 = tmp(T)
            nc.vector.tensor_reduce(out=m2[:],
                                    in_=_v(fval[:], 0, [[A, T], [1, A]]),
                                    axis=AX.X, op=OP.max)
            fmask = tmp(TA)
            nc.vector.tensor_tensor(out=_v(fmask[:], 0, [[A, T], [1, A]]),
                                    in0=_v(fval[:], 0, [[A, T], [1, A]]),
                                    in1=_v(m2[:], 0, [[1, T], [0, A]]),
                                    op=OP.is_equal)
            # fobj = fmask * obj (premasks every selected quantity)
            fobj = tmp(TA)
            nc.vector.tensor_tensor(out=_v(fobj[:], 0, [[A, T], [1, A]]),
                                    in0=_v(fmask[:], 0, [[A, T], [1, A]]),
                                    in1=_v(fp[:], O["OBJ"], [[1, T], [0, A]]),
                                    op=OP.mult)

            # cls path: se = sum_j e (bf16), lse via bf16 bit trick (gpsimd),
            # ce = lse - s -> SRC q6
            se = tmp(TA, bf16)
            with nc.allow_low_precision(reason="lse bit-trick needs bf16 bits; "
                                        "2e-2 loss tolerance"):
                nc.vector.tensor_reduce(
                    out=_v(se[:], 0, [[1, TA]]),
                    in_=_v(e[:], 0, [[NCLS, TA], [1, NCLS]]),
                    axis=AX.X, op=OP.add)
            lgf = tmp(TA)
            nc.gpsimd.tensor_copy(out=lgf[:], in_=se[:].bitcast(i16))
            lse = tmp(TA)
            nc.gpsimd.tensor_scalar(out=lse[:], in0=lgf[:],
                                    scalar1=LN2 / (1 << 7),
                                    scalar2=-LOG_BIAS * LN2,
                                    op0=OP.mult, op1=OP.add)
            nc.gpsimd.tensor_tensor(out=_v(SRC[:], 6 * TA, [[A, T], [1, A]]),
                                    in0=_v(lse[:], 0, [[A, T], [1, A]]),
                                    in1=_v(fp[:], O["SAUX"], [[A, T], [1, A]]),
                                    op=OP.subtract)

            # best-anchor selection of all seven quantities in one mul+reduce
            selm = pool.tile([P, 7 * TA], f32, name="selm")
            nc.vector.tensor_tensor(out=_v(selm[:], 0, [[TA, 7], [1, TA]]),
                                    in0=_v(SRC[:], 0, [[TA, 7], [1, TA]]),
                                    in1=_v(fobj[:], 0, [[0, 7], [1, TA]]),
                                    op=OP.mult)
            FIN = pool.tile([P, 7 * T], f32, name="FIN")
            nc.vector.tensor_reduce(out=_v(FIN[:], 0, [[T, 7], [1, T]]),
                                    in_=_v(selm[:], 0, [[TA, 7], [A, T], [1, A]]),
                                    axis=AX.X, op=OP.add)

            # box: FIN q0..3 -> oscl4*(sel - aux4)^2, written back into q0..3
            d4 = tmp(4 * T)
            nc.vector.tensor_tensor(out=d4[:], in0=_v(FIN[:], 0, [[1, 4 * T]]),
                                    in1=_v(fp[:], O["AUX4"], [[1, 4 * T]]),
                                    op=OP.subtract)
            d4m = tmp(4 * T)
            nc.vector.tensor_tensor(out=d4m[:], in0=d4[:],
                                    in1=_v(fp[:], O["OSCL4"], [[1, 4 * T]]),
                                    op=OP.mult)
            nc.vector.tensor_tensor(out=_v(FIN[:], 0, [[1, 4 * T]]),
                                    in0=d4[:], in1=d4m[:], op=OP.mult)

            # one grouped reduce -> partials cols 0..6
            nc.vector.tensor_reduce(out=_v(partials[:], 0, [[1, 7]]),
                                    in_=_v(FIN[:], 0, [[T, 7], [1, T]]),
                                    axis=AX.X, op=OP.add)

            nc.sync.dma_start(out=partials_d[:], in_=partials[:])

    if split:
        _split_multi_waits(nc)
    return nc


# -------------------------------------------------------------- shard builder
def _make_in_maps(out, gt_boxes, anchor_np, gt_classes_np, num_box_np, T):
    obj, xo, yo, tw, th, cls_t = _build_target_np(gt_boxes, gt_classes_np,
                                                  num_box_np)
    SLOTS = P * T
    TA = T * A
    out_r = out.reshape(B, A, 25, HWC)
    sqa = np.sqrt(anchor_np)                       # [A, 2]

    in_maps = []
    for c in range(CORES):
        sl = slice(c * BC, (c + 1) * BC)
        ob = obj[sl]                               # [BC, HWC]
        bloc, hwloc = np.nonzero(ob > 0)
        K = len(bloc)
        assert K <= SLOTS

        def place(vals):
            buf = np.zeros(SLOTS, dtype=np.float32)
            buf[:K] = vals
            return buf.reshape(P, T)

        objv = place(np.ones(K, dtype=np.float32))
        xov = place(xo[sl][bloc, hwloc])
        yov = place(yo[sl][bloc, hwloc])
        twv = place(tw[sl][bloc, hwloc])
        thv = place(th[sl][bloc, hwloc])

        # occupied-cell prediction channels [K, A, 25]
        colsb = np.zeros((SLOTS, A, 25), dtype=np.float32)
        if K:
            colsb[:K] = out_r[sl].transpose(0, 3, 1, 2)[bloc, hwloc]
        # cols_xw (t, ch{x,y,conf,w,h}, a)
        sel = colsb[:, :, [21, 22, 20, 23, 24]]            # (t, a, ch)
        cols_xw = np.ascontiguousarray(
            sel.reshape(P, T, A, 5).transpose(0, 1, 3, 2)).reshape(P, 25 * T)
        logits = np.ascontiguousarray(
            colsb[:, :, :20]).reshape(P, 100 * T)          # (t, a, j)

        # target-class logit per (t, a)
        clsv = place(cls_t[sl][bloc, hwloc].astype(np.float32)).astype(np.int64)
        s_aux = np.take_along_axis(
            colsb[:, :, :20].reshape(SLOTS, A, 20),
            clsv.reshape(SLOTS, 1, 1).repeat(A, axis=1), axis=2
        )[:, :, 0].reshape(P, TA).astype(np.float32)

        # xi-space target box edges (d{x,y}, t): center 2o-1, half-width t_wh
        cxv = 2.0 * xov - 1.0
        cyv = 2.0 * yov - 1.0
        b1 = np.stack([cxv - twv, cyv - thv], axis=1).reshape(P, 2 * T)
        b2 = np.stack([cxv + twv, cyv + thv], axis=1).reshape(P, 2 * T)
        tarea = (twv * thv).reshape(P, T)

        wconst = np.broadcast_to(A - np.arange(A, dtype=np.float32), (P, A))
        sqav = np.broadcast_to(sqa.reshape(1, 10), (P, 10))

        # AUX4 / OSCL4 in q-order (w, h, x, y)
        aux4 = np.stack([np.sqrt(twv), np.sqrt(thv), cxv, cyv],
                        axis=1).reshape(P, 4 * T)
        oscl4 = np.stack([objv, objv, 0.25 * objv, 0.25 * objv],
                         axis=1).reshape(P, 4 * T)

        fpack = np.concatenate(
            [cols_xw, b1, b2, tarea, wconst, sqav,
             s_aux, aux4, oscl4, objv.reshape(P, T)], axis=1)

        # dense conf channels: [BC, A, HWC] -> [P, 1280] bf16
        confd = out_r[sl][:, :, 20, :].reshape(P, -1)

        in_maps.append({
            "fpack": np.ascontiguousarray(fpack, dtype=np.float32),
            "lgpack": _bf16(logits),
            "confd": _bf16(confd),
        })
    return in_maps


# ---------------------------------------------------------------- entry point
def kernel(out, gt_boxes, anchor, gt_classes, num_box):
    from concourse.bass_utils import run_bass_kernel_spmd

    out = np.ascontiguousarray(np.asarray(out, dtype=np.float32))
    gt_boxes = np.asarray(gt_boxes, dtype=np.float32)
    anchor_np = np.asarray(anchor, dtype=np.float32)
    gt_classes_np = np.asarray(gt_classes)
    num_box_np = np.asarray(num_box)

    # per-core occupied-cell counts decide the compiled tile factor T
    obj = _build_target_np(gt_boxes, gt_classes_np, num_box_np)[0]
    ks = [int((obj[c * BC:(c + 1) * BC] > 0).sum()) for c in range(CORES)]
    maxk = max(ks)
    T = max(1, -(-maxk // P))
    assert maxk <= 13 * P and T <= 13

    in_maps = _make_in_maps(out, gt_boxes, anchor_np, gt_classes_np,
                            num_box_np, T)

    import os
    key = f"nc{T}"
    if key not in _CACHE:
        _CACHE[key] = _build_nc(T)
    trace = os.environ.get("KERNEL_TRACE", "0") == "1"
    res = run_bass_kernel_spmd(_CACHE[key], in_maps, core_ids=list(range(CORES)),
                               trace=trace)
    if trace:
        print(f"HW exec time: {res.exec_time_ns} ns  (mean {res.mean_exec_time_ns})")

    cols = np.zeros(12, dtype=np.float64)
    for c in range(CORES):
        cols += res.results[c]["partials"].astype(np.float64).sum(axis=0)
    K = float(sum(ks))
    box_loss = np.float32(LAM_COORD / B * (cols[0] + cols[1] + cols[2] + cols[3]))
    conf_loss = np.float32(LAM_OBJ / B * 0.25 * (cols[5] - 2.0 * cols[4] + K))
    nob_c = 0.25 * (cols[5] + 2.0 * cols[4] + K)
    dense = 0.25 * (float(B * A * HWC) + 2.0 * cols[7] + cols[8])
    noobj_loss = np.float32(LAM_NOOBJ / B * (dense - nob_c))
    cls_loss = np.float32(LAM_CLS / B * cols[6])
    return (box_loss, conf_loss, noobj_loss, cls_loss)


# revision 29
# speedup vs baseline: 1.0465x; 1.0094x over previous
"""Trainium2 Bass kernel for nn_Loss_65781719105930 (YOLO-style detection loss).

Strategy (pure data parallelism, 8 cores, 32 images each):
  host:   replicate the reference's target-build scatter (small int64 inputs),
          compact occupied cells, pre-pack aux tables + prediction columns into
          three contiguous DMA payloads; gather the target-class logit per
          (cell, anchor) host-side.
  device: dense pass over the 5 conf channels (sum of sigmoid^2), plus IoU /
          first-argmax / best-anchor-select / cross-entropy on compacted tiles.

Numeric tricks that keep the scalar engine on ONE activation-table set
(exp_and_others = {tanh, exp, square}):
  sigmoid(x)   = (1 + tanh(x/2)) / 2      -> work in xi = 2x-1 coords; the
                                             0.5 factors fold into host consts
  sqrt(exp(x)*anchor) = exp(x/2)*sqrt(anchor)
  ln(x)        ~ bitcast_i16(bf16 x) * ln2/2^7 - 126.94269504*ln2
                 (mean-centered log2 bit trick; loss tolerance is 2e-2 rel)

The grid offset cancels algebraically in both the IoU and the box loss, so it
never appears on device.

Device program layout: SRC [P, 7*T*A] holds quantities q = (w, h, x, y, u, uu,
ce) per (cell t, anchor a); one mul by (fmask*obj) + one reduce selects the
best anchor for all seven; one final grouped reduce produces all loss partial
sums at once.
"""
import numpy as np

# ---------------------------------------------------------------- constants
NCLS = 20
H = W = 32
HWC = H * W            # 1024 cells/image
A = 5
M = 50
B = 256
CORES = 8
BC = B // CORES        # 32 images per core
CH = A * (5 + NCLS)    # 125 channels
P = 128
LAM_COORD, LAM_OBJ, LAM_NOOBJ, LAM_CLS = 5.0, 1.0, 0.5, 1.0

LN2 = float(np.log(2.0))
LOG_BIAS = 126.94269504   # mean-centering constant for the log2 bit trick

_CACHE = {}


def _bf16(x):
    """float32 ndarray -> ml_dtypes.bfloat16 (RNE)."""
    import ml_dtypes
    return np.asarray(x, dtype=np.float32).astype(ml_dtypes.bfloat16)


# ---------------------------------------------------------------- host prep
def _build_target_np(gt_boxes, gt_classes, num_box):
    """Numpy replication of reference.build_target (last object wins, first-max
    class argmax). Returns per-cell [B, HWC] arrays."""
    Bn = gt_boxes.shape[0]
    valid = np.arange(M)[None, :] < num_box[:, None]
    x = gt_boxes[..., 0].astype(np.float32) * H
    y = gt_boxes[..., 1].astype(np.float32) * H
    gx = np.floor(x).astype(np.int64)
    gy = np.floor(y).astype(np.int64)
    flat = np.where(valid, gy * W + gx, HWC)
    bi = np.broadcast_to(np.arange(Bn)[:, None], (Bn, M))

    vals = np.stack([np.ones_like(x), x - gx, y - gy,
                     gt_boxes[..., 2].astype(np.float32) * H,
                     gt_boxes[..., 3].astype(np.float32) * H], axis=-1)
    tgt_box = np.zeros((Bn, HWC + 1, 5), dtype=np.float32)
    tgt_box[bi, flat] = vals
    tgt_cls = np.zeros((Bn, HWC + 1, NCLS), dtype=np.float32)
    tgt_cls[bi, flat, gt_classes.astype(np.int64)] = 1.0

    tgt_box = tgt_box[:, :HWC]
    obj = tgt_box[..., 0]
    cls_t = np.argmax(tgt_cls[:, :HWC], axis=-1).astype(np.int64)
    return obj, tgt_box[..., 1], tgt_box[..., 2], tgt_box[..., 3], tgt_box[..., 4], cls_t


def _split_multi_waits(nc):
    """This container's walrus accepts only ONE sem-wait per instruction; hoist
    extra waits onto standalone NoOps."""
    import concourse.mybir as mybir
    import bass_rust
    n = 0
    for fn in nc.m.functions:
        for blk in fn.blocks:
            new = []
            for ins in blk.instructions:
                si = ins.sync_info
                waits = list(si.on_wait) if si is not None else []
                if len(waits) > 1:
                    for w in waits[:-1]:
                        nop = mybir.InstNoOp(name=f"{ins.name}-w{n}")
                        nop.engine = ins.engine
                        nop.sync_info = bass_rust.SyncInfo(on_wait=[w], on_update=[])
                        new.append(nop)
                        n += 1
                    si.on_wait = [waits[-1]]
                    ins.sync_info = si
                new.append(ins)
            blk.instructions = new
    return n


def _offsets(T):
    """fpack free-dim offsets. cols_xw channel order is (x, y, conf, w, h)."""
    o = {}
    o["XW"] = 0
    o["B1"] = 25 * T
    o["B2"] = o["B1"] + 2 * T
    o["TAREA"] = o["B2"] + 2 * T
    o["WCONST"] = o["TAREA"] + T
    o["SQA"] = o["WCONST"] + 5 * T
    o["SAUX"] = o["SQA"] + 10
    o["AUX4"] = o["SAUX"] + 5 * T
    o["OSCL4"] = o["AUX4"] + 4 * T
    o["OBJ"] = o["OSCL4"] + 4 * T
    o["NF"] = o["OBJ"] + T
    return o


# ---------------------------------------------------------------- bass build
def _build_nc(T, split=True):
    """Build the per-core kernel for T cell-blocks per partition (P*T slots).

    fpack [P, NF] f32:
      cols_xw (t, ch{x,y,conf,w,h}, a)  25T
      B1, B2  (d{x,y}, t)               2T each   xi-space target box edges
      TAREA   (t)                       T         tw*th (physical, cell units)
      WCONST  (a)                       5         A - a (first-argmax tiebreak)
      SQA     (d{w,h}, a)               10        sqrt(anchor)
      S_AUX   (t, a)                    5T        target-class logit
      AUX4    (q{w,h,x,y}, t)           4T        (sqrt tw, sqrt th, 2xo-1, 2yo-1)
      OSCL4   (q, t)                    4T        obj * {1,1,.25,.25}
      OBJ     (t)                       T
    lgpack bf16 [P, 100T]: logits (t, a, j)
    confd  bf16 [P, 1280]: all conf channels (dense noobj pass)
    partials out [P, 12]:
      0..3 box (w,h,x,y) sq-diff sums, 4 sum obj*u, 5 sum obj*u^2,
      6 sum obj*ce, 7 dense sum ud, 8 dense sum ud^2   (u/ud = tanh(conf/2))
    """
    import concourse.bass as bass
    import concourse.mybir as mybir
    import concourse.tile as tile

    f32 = mybir.dt.float32
    bf16 = mybir.dt.bfloat16
    i16 = mybir.dt.int16
    AF = mybir.ActivationFunctionType
    OP = mybir.AluOpType
    AX = mybir.AxisListType

    TA = T * A
    TA2 = TA * 2
    O = _offsets(T)
    NF = O["NF"]
    DF = BC * A * HWC // P   # 1280 dense conf elements per partition

    def _v(ap, off, dims):
        """Sub-view of a tile AP: keep its partition dim, replace free dims."""
        return bass.AP(tensor=ap.tensor, offset=ap.offset + off,
                       ap=[list(ap.ap[0])] + dims)

    nc = bass.Bass("TRN2")
    fpack_d = nc.declare_dram_parameter("fpack", [P, NF], f32, isOutput=False)
    lgpack_d = nc.declare_dram_parameter("lgpack", [P, 100 * T], bf16, isOutput=False)
    confd_d = nc.declare_dram_parameter("confd", [P, DF], bf16, isOutput=False)
    partials_d = nc.declare_dram_parameter("partials", [P, 12], f32, isOutput=True)

    with tile.TileContext(nc) as tc:
        with tc.tile_pool(name="sb", bufs=1) as pool:
            # ---------------- input DMAs, priority order, on sync HWDGE
            fp = pool.tile([P, NF], f32, name="fp")
            nc.sync.dma_start(out=fp[:], in_=fpack_d[:])
            lg_in = pool.tile([P, 100 * T], bf16, name="lg_in")
            nc.sync.dma_start(out=lg_in[:], in_=lgpack_d[:])
            confd = pool.tile([P, DF], bf16, name="confd")
            nc.sync.dma_start(out=confd[:], in_=confd_d[:])

            partials = pool.tile([P, 12], f32, name="partials")

            # SRC: (q, t, a) with q in {w, h, x, y, u, uu, ce}
            SRC = pool.tile([P, 7 * TA], f32, name="SRC")

            # ---------------- scalar engine program (one act-table set)
            # EW = exp(chan{w,h}/2): chans 3,4 of cols_xw -> (t, a, d)
            EW = pool.tile([P, TA2], f32, name="EW")
            nc.scalar.activation(
                _v(EW[:], 0, [[1, 2], [2 * A, T], [2, A]]),
                _v(fp[:], O["XW"] + 15, [[5, 2], [25, T], [1, A]]),
                AF.Exp, scale=0.5)
            # x, y, u = tanh(chan{x,y,conf}/2) -> SRC q2, q3, q4
            nc.scalar.activation(
                _v(SRC[:], 2 * TA, [[TA, 3], [A, T], [1, A]]),
                _v(fp[:], O["XW"], [[5, 3], [25, T], [1, A]]),
                AF.Tanh, scale=0.5)
            # e = exp(logits), bf16, (t, a, j)
            e = pool.tile([P, 100 * T], bf16, name="e")
            nc.scalar.activation(
                _v(e[:], 0, [[100, T], [NCLS, A], [1, NCLS]]),
                _v(lg_in[:], 0, [[100, T], [NCLS, A], [1, NCLS]]),
                AF.Exp)
            # dense noobj pass: ud = tanh(c/2); sum sigma^2 = 0.25*(N + 2*sum ud
            # + sum ud^2)
            # uu = u^2 -> SRC q5 (square is in the same act-table set)
            nc.scalar.activation(_v(SRC[:], 5 * TA, [[1, TA]]),
                                 _v(SRC[:], 4 * TA, [[1, TA]]), AF.Square)
            UD = pool.tile([P, DF], f32, name="UD")
            nc.scalar.activation(UD[:], confd[:], AF.Tanh, scale=0.5,
                                 accum_out=_v(partials[:], 7, [[1, 1]]))
            SQD = pool.tile([P, DF], f32, name="SQD")
            nc.scalar.activation(SQD[:], UD[:], AF.Square,
                                 accum_out=_v(partials[:], 8, [[1, 1]]))

            # ---------------- vector engine program
            tcnt = [0]

            def tmp(n, dtype=f32):
                tcnt[0] += 1
                return pool.tile([P, n], dtype, name=f"t{tcnt[0]}")

            # sh = EW * sqrt(anchor) = sqrt(pred_wh) -> SRC q0, q1
            SH = _v(SRC[:], 0, [[TA, 2], [A, T], [1, A]])
            nc.vector.tensor_tensor(
                out=SH,
                in0=_v(EW[:], 0, [[1, 2], [2 * A, T], [2, A]]),
                in1=_v(fp[:], O["SQA"], [[1, 2], [0, T], [2, A]]),
                op=OP.mult)
            # wfull = sh*sh = pred_wh (xi-space half-width), (d, t, a)
            wf = tmp(TA2)
            SH2 = _v(SRC[:], 0, [[TA, 2], [1, TA]])
            WF = _v(wf[:], 0, [[TA, 2], [1, TA]])
            nc.vector.tensor_tensor(out=WF, in0=SH2, in1=SH2, op=OP.mult)

            # IoU in xi coords. XY = SRC q2, q3 as (d, t, a)
            XY = _v(SRC[:], 2 * TA, [[TA, 2], [1, TA]])
            lo = tmp(TA2)
            nc.vector.tensor_tensor(out=lo[:], in0=XY, in1=WF, op=OP.subtract)
            hi = tmp(TA2)
            nc.vector.tensor_tensor(out=hi[:], in0=XY, in1=WF, op=OP.add)
            B1v = _v(fp[:], O["B1"], [[T, 2], [1, T], [0, A]])
            B2v = _v(fp[:], O["B2"], [[T, 2], [1, T], [0, A]])
            LOv = _v(lo[:], 0, [[TA, 2], [A, T], [1, A]])
            HIv = _v(hi[:], 0, [[TA, 2], [A, T], [1, A]])
            t1 = tmp(TA2)
            nc.vector.tensor_tensor(out=_v(t1[:], 0, [[TA, 2], [A, T], [1, A]]),
                                    in0=HIv, in1=B2v, op=OP.min)
            t2 = tmp(TA2)
            nc.vector.tensor_tensor(out=_v(t2[:], 0, [[TA, 2], [A, T], [1, A]]),
                                    in0=LOv, in1=B1v, op=OP.max)
            t3 = tmp(TA2)
            nc.vector.tensor_tensor(out=t3[:], in0=t1[:], in1=t2[:], op=OP.subtract)
            # iw = max(t3, 0) * 0.5 -> physical overlap widths (d, t, a)
            iwih = tmp(TA2)
            nc.vector.tensor_scalar(out=iwih[:], in0=t3[:], scalar1=0.0,
                                    scalar2=0.5, op0=OP.max, op1=OP.mult)
            inter = tmp(TA)
            nc.vector.tensor_tensor(out=inter[:], in0=_v(iwih[:], 0, [[1, TA]]),
                                    in1=_v(iwih[:], TA, [[1, TA]]), op=OP.mult)
            # union side branch on gpsimd, in parallel with the inter chain
            areaA = tmp(TA)
            nc.gpsimd.tensor_tensor(out=areaA[:], in0=_v(wf[:], 0, [[1, TA]]),
                                    in1=_v(wf[:], TA, [[1, TA]]), op=OP.mult)
            u1 = tmp(TA)
            nc.gpsimd.tensor_tensor(out=_v(u1[:], 0, [[A, T], [1, A]]),
                                    in0=_v(areaA[:], 0, [[A, T], [1, A]]),
                                    in1=_v(fp[:], O["TAREA"], [[1, T], [0, A]]),
                                    op=OP.add)
            u2 = tmp(TA)
            nc.vector.tensor_tensor(out=u2[:], in0=u1[:], in1=inter[:],
                                    op=OP.subtract)
            rcp = tmp(TA)
            nc.vector.reciprocal(out=rcp[:], in_=u2[:])
            iou = tmp(TA)
            nc.vector.tensor_tensor(out=iou[:], in0=inter[:], in1=rcp[:],
                                    op=OP.mult)

            # first-argmax over a -> fmask (exact float equality + tiebreak)
            rmax = tmp(T)
            nc.vector.tensor_reduce(out=rmax[:],
                                    in_=_v(iou[:], 0, [[A, T], [1, A]]),
                                    axis=AX.X, op=OP.max)
            eq = tmp(TA)
            nc.vector.tensor_tensor(out=_v(eq[:], 0, [[A, T], [1, A]]),
                                    in0=_v(iou[:], 0, [[A, T], [1, A]]),
                                    in1=_v(rmax[:], 0, [[1, T], [0, A]]),
                                    op=OP.is_equal)
            # WOBJ = (A - a) * obj: padding cells give fval = 0 -> fmask all-1
            # there, which is harmless: u/uu/ce/box are all zero at padding.
            fval = tmp(TA)
            nc.vector.tensor_tensor(out=_v(fval[:], 0, [[A, T], [1, A]]),
                                    in0=_v(eq[:], 0, [[A, T], [1, A]]),
                                    in1=_v(fp[:], O["WCONST"], [[A, T], [1, A]]),
                                    op=OP.mult)
            m2 = tmp(T)
            nc.vector.tensor_reduce(out=m2[:],
                                    in_=_v(fval[:], 0, [[A, T], [1, A]]),
                                    axis=AX.X, op=OP.max)
            fmask = tmp(TA)
            nc.vector.tensor_tensor(out=_v(fmask[:], 0, [[A, T], [1, A]]),
                                    in0=_v(fval[:], 0, [[A, T], [1, A]]),
                                    in1=_v(m2[:], 0, [[1, T], [0, A]]),
                                    op=OP.is_equal)
            # cls path: se = sum_j e (bf16), lse via bf16 bit trick (gpsimd),
            # ce = lse - s -> SRC q6
            se = tmp(TA, bf16)
            with nc.allow_low_precision(reason="lse bit-trick needs bf16 bits; "
                                        "2e-2 loss tolerance"):
                nc.vector.tensor_reduce(
                    out=_v(se[:], 0, [[1, TA]]),
                    in_=_v(e[:], 0, [[NCLS, TA], [1, NCLS]]),
                    axis=AX.X, op=OP.add)
            lgf = tmp(TA)
            nc.gpsimd.tensor_copy(out=lgf[:], in_=se[:].bitcast(i16))
            lse = tmp(TA)
            nc.gpsimd.tensor_scalar(out=lse[:], in0=lgf[:],
                                    scalar1=LN2 / (1 << 7),
                                    scalar2=-LOG_BIAS * LN2,
                                    op0=OP.mult, op1=OP.add)
            nc.gpsimd.tensor_tensor(out=_v(SRC[:], 6 * TA, [[A, T], [1, A]]),
                                    in0=_v(lse[:], 0, [[A, T], [1, A]]),
                                    in1=_v(fp[:], O["SAUX"], [[A, T], [1, A]]),
                                    op=OP.subtract)

            # best-anchor selection of all seven quantities in one mul+reduce
            selm = pool.tile([P, 7 * TA], f32, name="selm")
            nc.vector.tensor_tensor(out=_v(selm[:], 0, [[TA, 7], [1, TA]]),
                                    in0=_v(SRC[:], 0, [[TA, 7], [1, TA]]),
                                    in1=_v(fmask[:], 0, [[0, 7], [1, TA]]),
                                    op=OP.mult)
            FIN = pool.tile([P, 7 * T], f32, name="FIN")
            nc.vector.tensor_reduce(out=_v(FIN[:], 0, [[T, 7], [1, T]]),
                                    in_=_v(selm[:], 0, [[TA, 7], [A, T], [1, A]]),
                                    axis=AX.X, op=OP.add)

            # box: FIN q0..3 -> oscl4*(sel - aux4)^2, written back into q0..3
            d4 = tmp(4 * T)
            nc.vector.tensor_tensor(out=d4[:], in0=_v(FIN[:], 0, [[1, 4 * T]]),
                                    in1=_v(fp[:], O["AUX4"], [[1, 4 * T]]),
                                    op=OP.subtract)
            d4m = tmp(4 * T)
            nc.vector.tensor_tensor(out=d4m[:], in0=d4[:],
                                    in1=_v(fp[:], O["OSCL4"], [[1, 4 * T]]),
                                    op=OP.mult)
            nc.vector.tensor_tensor(out=_v(FIN[:], 0, [[1, 4 * T]]),
                                    in0=d4[:], in1=d4m[:], op=OP.mult)

            # one grouped reduce -> partials cols 0..6
            nc.vector.tensor_reduce(out=_v(partials[:], 0, [[1, 7]]),
                                    in_=_v(FIN[:], 0, [[T, 7], [1, T]]),
                                    axis=AX.X, op=OP.add)

            nc.sync.dma_start(out=partials_d[:], in_=partials[:])

    if split:
        _split_multi_waits(nc)
    return nc


# -------------------------------------------------------------- shard builder
def _make_in_maps(out, gt_boxes, anchor_np, gt_classes_np, num_box_np, T):
    obj, xo, yo, tw, th, cls_t = _build_target_np(gt_boxes, gt_classes_np,
                                                  num_box_np)
    SLOTS = P * T
    TA = T * A
    out_r = out.reshape(B, A, 25, HWC)
    sqa = np.sqrt(anchor_np)                       # [A, 2]

    in_maps = []
    for c in range(CORES):
        sl = slice(c * BC, (c + 1) * BC)
        ob = obj[sl]                               # [BC, HWC]
        bloc, hwloc = np.nonzero(ob > 0)
        K = len(bloc)
        assert K <= SLOTS

        def place(vals):
            buf = np.zeros(SLOTS, dtype=np.float32)
            buf[:K] = vals
            return buf.reshape(P, T)

        objv = place(np.ones(K, dtype=np.float32))
        xov = place(xo[sl][bloc, hwloc])
        yov = place(yo[sl][bloc, hwloc])
        twv = place(tw[sl][bloc, hwloc])
        thv = place(th[sl][bloc, hwloc])

        # occupied-cell prediction channels [K, A, 25]
        colsb = np.zeros((SLOTS, A, 25), dtype=np.float32)
        if K:
            colsb[:K] = out_r[sl].transpose(0, 3, 1, 2)[bloc, hwloc]
        # cols_xw (t, ch{x,y,conf,w,h}, a)
        sel = colsb[:, :, [21, 22, 20, 23, 24]]            # (t, a, ch)
        cols_xw = np.ascontiguousarray(
            sel.reshape(P, T, A, 5).transpose(0, 1, 3, 2)).reshape(P, 25 * T)
        logits = np.ascontiguousarray(
            colsb[:, :, :20]).reshape(P, 100 * T)          # (t, a, j)

        # target-class logit per (t, a); padding slots get the exact device
        # lse of se=20.0 (logits 0 -> e all 1) so padded ce comes out 0
        clsv = place(cls_t[sl][bloc, hwloc].astype(np.float32)).astype(np.int64)
        s_aux = np.take_along_axis(
            colsb[:, :, :20].reshape(SLOTS, A, 20),
            clsv.reshape(SLOTS, 1, 1).repeat(A, axis=1), axis=2
        )[:, :, 0].astype(np.float32)
        i16pad = float(_bf16(np.float32(20.0)).view(np.int16))
        lse_pad = np.float32(np.float32(i16pad * np.float32(LN2 / (1 << 7)))
                             + np.float32(-LOG_BIAS * LN2))
        s_aux[K:] = lse_pad
        s_aux = s_aux.reshape(P, TA)

        # xi-space target box edges (d{x,y}, t): center 2o-1, half-width t_wh
        cxv = 2.0 * xov - 1.0
        cyv = 2.0 * yov - 1.0
        b1 = np.stack([cxv - twv, cyv - thv], axis=1).reshape(P, 2 * T)
        b2 = np.stack([cxv + twv, cyv + thv], axis=1).reshape(P, 2 * T)
        tarea = (twv * thv).reshape(P, T)

        # (A - a) * obj in (t, a) layout: padding cells get all-zero fval
        wconst = np.ascontiguousarray(
            (A - np.arange(A, dtype=np.float32))[None, None, :]
            * objv[:, :, None]).reshape(P, 5 * T)
        sqav = np.broadcast_to(sqa.reshape(1, 10), (P, 10))

        # AUX4 / OSCL4 in q-order (w, h, x, y)
        aux4 = np.stack([np.sqrt(twv), np.sqrt(thv), cxv, cyv],
                        axis=1).reshape(P, 4 * T)
        oscl4 = np.stack([objv, objv, 0.25 * objv, 0.25 * objv],
                         axis=1).reshape(P, 4 * T)

        fpack = np.concatenate(
            [cols_xw, b1, b2, tarea, wconst, sqav,
             s_aux, aux4, oscl4, objv.reshape(P, T)], axis=1)

        # dense conf channels: [BC, A, HWC] -> [P, 1280] bf16
        confd = out_r[sl][:, :, 20, :].reshape(P, -1)

        in_maps.append({
            "fpack": np.ascontiguousarray(fpack, dtype=np.float32),
            "lgpack": _bf16(logits),
            "confd": _bf16(confd),
        })
    return in_maps


# ---------------------------------------------------------------- entry point
def kernel(out, gt_boxes, anchor, gt_classes, num_box):
    from concourse.bass_utils import run_bass_kernel_spmd

    out = np.ascontiguousarray(np.asarray(out, dtype=np.float32))
    gt_boxes = np.asarray(gt_boxes, dtype=np.float32)
    anchor_np = np.asarray(anchor, dtype=np.float32)
    gt_classes_np = np.asarray(gt_classes)
    num_box_np = np.asarray(num_box)

    # per-core occupied-cell counts decide the compiled tile factor T
    obj = _build_target_np(gt_boxes, gt_classes_np, num_box_np)[0]
    ks = [int((obj[c * BC:(c + 1) * BC] > 0).sum()) for c in range(CORES)]
    maxk = max(ks)
    T = max(1, -(-maxk // P))
    assert maxk <= 13 * P and T <= 13

    in_maps = _make_in_maps(out, gt_boxes, anchor_np, gt_classes_np,
                            num_box_np, T)

    import os
    key = f"nc{T}"
    if key not in _CACHE:
        _CACHE[key] = _build_nc(T)
    trace = os.environ.get("KERNEL_TRACE", "0") == "1"
    res = run_bass_kernel_spmd(_CACHE[key], in_maps, core_ids=list(range(CORES)),
                               trace=trace)
    if trace:
        print(f"HW exec time: {res.exec_time_ns} ns  (mean {res.mean_exec_time_ns})")

    cols = np.zeros(12, dtype=np.float64)
    for c in range(CORES):
        cols += res.results[c]["partials"].astype(np.float64).sum(axis=0)
    K = float(sum(ks))
    box_loss = np.float32(LAM_COORD / B * (cols[0] + cols[1] + cols[2] + cols[3]))
    conf_loss = np.float32(LAM_OBJ / B * 0.25 * (cols[5] - 2.0 * cols[4] + K))
    nob_c = 0.25 * (cols[5] + 2.0 * cols[4] + K)
    dense = 0.25 * (float(B * A * HWC) + 2.0 * cols[7] + cols[8])
    noobj_loss = np.float32(LAM_NOOBJ / B * (dense - nob_c))
    cls_loss = np.float32(LAM_CLS / B * cols[6])
    return (box_loss, conf_loss, noobj_loss, cls_loss)


# revision 35
# speedup vs baseline: 1.0553x; 1.0084x over previous
"""Trainium2 Bass kernel for nn_Loss_65781719105930 (YOLO-style detection loss).

Strategy (pure data parallelism, 8 cores, 32 images each):
  host:   replicate the reference's target-build scatter (small int64 inputs),
          compact occupied cells, pre-pack aux tables + prediction columns into
          three contiguous DMA payloads; gather the target-class logit per
          (cell, anchor) host-side.
  device: dense pass over the 5 conf channels (sum of sigmoid^2), plus IoU /
          first-argmax / best-anchor-select / cross-entropy on compacted tiles.

Numeric tricks that keep the scalar engine on ONE activation-table set
(exp_and_others = {tanh, exp, square}):
  sigmoid(x)   = (1 + tanh(x/2)) / 2      -> work in xi = 2x-1 coords; the
                                             0.5 factors fold into host consts
  sqrt(exp(x)*anchor) = exp(x/2)*sqrt(anchor)
  ln(x)        ~ bitcast_i16(bf16 x) * ln2/2^7 - 126.94269504*ln2
                 (mean-centered log2 bit trick; loss tolerance is 2e-2 rel)

The grid offset cancels algebraically in both the IoU and the box loss, so it
never appears on device.

Device program layout: SRC [P, 7*T*A] holds quantities q = (w, h, x, y, u, uu,
ce) per (cell t, anchor a); one mul by (fmask*obj) + one reduce selects the
best anchor for all seven; one final grouped reduce produces all loss partial
sums at once.
"""
import numpy as np

# ---------------------------------------------------------------- constants
NCLS = 20
H = W = 32
HWC = H * W            # 1024 cells/image
A = 5
M = 50
B = 256
CORES = 8
BC = B // CORES        # 32 images per core
CH = A * (5 + NCLS)    # 125 channels
P = 128
LAM_COORD, LAM_OBJ, LAM_NOOBJ, LAM_CLS = 5.0, 1.0, 0.5, 1.0

LN2 = float(np.log(2.0))
LOG_BIAS = 126.94269504   # mean-centering constant for the log2 bit trick

_CACHE = {}


def _bf16(x):
    """float32 ndarray -> ml_dtypes.bfloat16 (RNE)."""
    import ml_dtypes
    return np.asarray(x, dtype=np.float32).astype(ml_dtypes.bfloat16)


# ---------------------------------------------------------------- host prep
def _build_target_np(gt_boxes, gt_classes, num_box):
    """Numpy replication of reference.build_target (last object wins, first-max
    class argmax). Returns per-cell [B, HWC] arrays."""
    Bn = gt_boxes.shape[0]
    valid = np.arange(M)[None, :] < num_box[:, None]
    x = gt_boxes[..., 0].astype(np.float32) * H
    y = gt_boxes[..., 1].astype(np.float32) * H
    gx = np.floor(x).astype(np.int64)
    gy = np.floor(y).astype(np.int64)
    flat = np.where(valid, gy * W + gx, HWC)
    bi = np.broadcast_to(np.arange(Bn)[:, None], (Bn, M))

    vals = np.stack([np.ones_like(x), x - gx, y - gy,
                     gt_boxes[..., 2].astype(np.float32) * H,
                     gt_boxes[..., 3].astype(np.float32) * H], axis=-1)
    tgt_box = np.zeros((Bn, HWC + 1, 5), dtype=np.float32)
    tgt_box[bi, flat] = vals
    tgt_cls = np.zeros((Bn, HWC + 1, NCLS), dtype=np.float32)
    tgt_cls[bi, flat, gt_classes.astype(np.int64)] = 1.0

    tgt_box = tgt_box[:, :HWC]
    obj = tgt_box[..., 0]
    cls_t = np.argmax(tgt_cls[:, :HWC], axis=-1).astype(np.int64)
    return obj, tgt_box[..., 1], tgt_box[..., 2], tgt_box[..., 3], tgt_box[..., 4], cls_t


def _split_multi_waits(nc):
    """This container's walrus accepts only ONE sem-wait per instruction; hoist
    extra waits onto standalone NoOps."""
    import concourse.mybir as mybir
    import bass_rust
    n = 0
    for fn in nc.m.functions:
        for blk in fn.blocks:
            new = []
            for ins in blk.instructions:
                si = ins.sync_info
                waits = list(si.on_wait) if si is not None else []
                if len(waits) > 1:
                    for w in waits[:-1]:
                        nop = mybir.InstNoOp(name=f"{ins.name}-w{n}")
                        nop.engine = ins.engine
                        nop.sync_info = bass_rust.SyncInfo(on_wait=[w], on_update=[])
                        new.append(nop)
                        n += 1
                    si.on_wait = [waits[-1]]
                    ins.sync_info = si
                new.append(ins)
            blk.instructions = new
    return n


def _offsets(T):
    """fpack free-dim offsets. cols_xw channel order is (x, y, conf, w, h)."""
    o = {}
    o["XW"] = 0
    o["B1"] = 25 * T
    o["B2"] = o["B1"] + 2 * T
    o["TAREA"] = o["B2"] + 2 * T
    o["WCONST"] = o["TAREA"] + T
    o["SQA"] = o["WCONST"] + 5 * T
    o["SAUX"] = o["SQA"] + 10
    o["AUX4"] = o["SAUX"] + 5 * T
    o["OSCL4"] = o["AUX4"] + 4 * T
    o["OBJ"] = o["OSCL4"] + 4 * T
    o["NF"] = o["OBJ"] + T
    return o


# ---------------------------------------------------------------- bass build
def _build_nc(T, split=True):
    """Build the per-core kernel for T cell-blocks per partition (P*T slots).

    fpack [P, NF] f32:
      cols_xw (t, ch{x,y,conf,w,h}, a)  25T
      B1, B2  (d{x,y}, t)               2T each   xi-space target box edges
      TAREA   (t)                       T         tw*th (physical, cell units)
      WCONST  (a)                       5         A - a (first-argmax tiebreak)
      SQA     (d{w,h}, a)               10        sqrt(anchor)
      S_AUX   (t, a)                    5T        target-class logit
      AUX4    (q{w,h,x,y}, t)           4T        (sqrt tw, sqrt th, 2xo-1, 2yo-1)
      OSCL4   (q, t)                    4T        obj * {1,1,.25,.25}
      OBJ     (t)                       T
    lgpack bf16 [P, 100T]: logits (t, a, j)
    confd  bf16 [P, 1280]: all conf channels (dense noobj pass)
    partials out [P, 12]:
      0..3 box (w,h,x,y) sq-diff sums, 4 sum obj*u, 5 sum obj*u^2,
      6 sum obj*ce, 7 dense sum ud, 8 dense sum ud^2   (u/ud = tanh(conf/2))
    """
    import concourse.bass as bass
    import concourse.mybir as mybir
    import concourse.tile as tile

    f32 = mybir.dt.float32
    bf16 = mybir.dt.bfloat16
    i16 = mybir.dt.int16
    AF = mybir.ActivationFunctionType
    OP = mybir.AluOpType
    AX = mybir.AxisListType

    TA = T * A
    TA2 = TA * 2
    O = _offsets(T)
    NF = O["NF"]
    DF = BC * A * HWC // P   # 1280 dense conf elements per partition

    def _v(ap, off, dims):
        """Sub-view of a tile AP: keep its partition dim, replace free dims."""
        return bass.AP(tensor=ap.tensor, offset=ap.offset + off,
                       ap=[list(ap.ap[0])] + dims)

    nc = bass.Bass("TRN2")
    fpack_d = nc.declare_dram_parameter("fpack", [P, NF], f32, isOutput=False)
    lgpack_d = nc.declare_dram_parameter("lgpack", [P, 100 * T], bf16, isOutput=False)
    confd_d = nc.declare_dram_parameter("confd", [P, DF], bf16, isOutput=False)
    partials_d = nc.declare_dram_parameter("partials", [P, 12], f32, isOutput=True)

    with tile.TileContext(nc) as tc:
        with tc.tile_pool(name="sb", bufs=1) as pool:
            # ---------------- input DMAs, priority order, on sync HWDGE
            fp = pool.tile([P, NF], f32, name="fp")
            nc.sync.dma_start(out=fp[:], in_=fpack_d[:])
            lg_in = pool.tile([P, 100 * T], bf16, name="lg_in")
            nc.sync.dma_start(out=lg_in[:], in_=lgpack_d[:])
            confd = pool.tile([P, DF], bf16, name="confd")
            nc.sync.dma_start(out=confd[:], in_=confd_d[:])

            partials = pool.tile([P, 12], f32, name="partials")

            # SRC: (q, t, a) with q in {w, h, x, y, u, uu, ce}
            SRC = pool.tile([P, 7 * TA], f32, name="SRC")

            # ---------------- scalar engine program (one act-table set)
            # EW = exp(chan{w,h}/2): chans 3,4 of cols_xw -> (t, a, d)
            EW = pool.tile([P, TA2], f32, name="EW")
            nc.scalar.activation(
                _v(EW[:], 0, [[1, 2], [2 * A, T], [2, A]]),
                _v(fp[:], O["XW"] + 15, [[5, 2], [25, T], [1, A]]),
                AF.Exp, scale=0.5)
            # x, y, u = tanh(chan{x,y,conf}/2) -> SRC q2, q3, q4
            nc.scalar.activation(
                _v(SRC[:], 2 * TA, [[TA, 3], [A, T], [1, A]]),
                _v(fp[:], O["XW"], [[5, 3], [25, T], [1, A]]),
                AF.Tanh, scale=0.5)
            # e = exp(logits), bf16, (t, a, j)
            e = pool.tile([P, 100 * T], bf16, name="e")
            nc.scalar.activation(
                _v(e[:], 0, [[100, T], [NCLS, A], [1, NCLS]]),
                _v(lg_in[:], 0, [[100, T], [NCLS, A], [1, NCLS]]),
                AF.Exp)
            # dense noobj pass: ud = tanh(c/2); sum sigma^2 = 0.25*(N + 2*sum ud
            # + sum ud^2)
            # uu = u^2 -> SRC q5 (square is in the same act-table set)
            nc.scalar.activation(_v(SRC[:], 5 * TA, [[1, TA]]),
                                 _v(SRC[:], 4 * TA, [[1, TA]]), AF.Square)
            UD = pool.tile([P, DF], f32, name="UD")
            nc.scalar.activation(UD[:], confd[:], AF.Tanh, scale=0.5,
                                 accum_out=_v(partials[:], 7, [[1, 1]]))
            SQD = pool.tile([P, DF], f32, name="SQD")
            nc.scalar.activation(SQD[:], UD[:], AF.Square,
                                 accum_out=_v(partials[:], 8, [[1, 1]]))

            # ---------------- vector engine program
            tcnt = [0]

            def tmp(n, dtype=f32):
                tcnt[0] += 1
                return pool.tile([P, n], dtype, name=f"t{tcnt[0]}")

            # sh = EW * sqrt(anchor) = sqrt(pred_wh) -> SRC q0, q1
            SH = _v(SRC[:], 0, [[TA, 2], [A, T], [1, A]])
            nc.vector.tensor_tensor(
                out=SH,
                in0=_v(EW[:], 0, [[1, 2], [2 * A, T], [2, A]]),
                in1=_v(fp[:], O["SQA"], [[1, 2], [0, T], [2, A]]),
                op=OP.mult)
            # wfull = sh*sh = pred_wh (xi-space half-width), (d, t, a)
            wf = tmp(TA2)
            SH2 = _v(SRC[:], 0, [[TA, 2], [1, TA]])
            WF = _v(wf[:], 0, [[TA, 2], [1, TA]])
            nc.vector.tensor_tensor(out=WF, in0=SH2, in1=SH2, op=OP.mult)

            # IoU in xi coords. XY = SRC q2, q3 as (d, t, a)
            XY = _v(SRC[:], 2 * TA, [[TA, 2], [1, TA]])
            lo = tmp(TA2)
            nc.vector.tensor_tensor(out=lo[:], in0=XY, in1=WF, op=OP.subtract)
            hi = tmp(TA2)
            nc.vector.tensor_tensor(out=hi[:], in0=XY, in1=WF, op=OP.add)
            B1v = _v(fp[:], O["B1"], [[T, 2], [1, T], [0, A]])
            B2v = _v(fp[:], O["B2"], [[T, 2], [1, T], [0, A]])
            LOv = _v(lo[:], 0, [[TA, 2], [A, T], [1, A]])
            HIv = _v(hi[:], 0, [[TA, 2], [A, T], [1, A]])
            t1 = tmp(TA2)
            nc.vector.tensor_tensor(out=_v(t1[:], 0, [[TA, 2], [A, T], [1, A]]),
                                    in0=HIv, in1=B2v, op=OP.min)
            t2 = tmp(TA2)
            nc.vector.tensor_tensor(out=_v(t2[:], 0, [[TA, 2], [A, T], [1, A]]),
                                    in0=LOv, in1=B1v, op=OP.max)
            t3 = tmp(TA2)
            nc.vector.tensor_tensor(out=t3[:], in0=t1[:], in1=t2[:], op=OP.subtract)
            # iw = max(t3, 0) * 0.5 -> physical overlap widths (d, t, a)
            iwih = tmp(TA2)
            nc.vector.tensor_scalar(out=iwih[:], in0=t3[:], scalar1=0.0,
                                    scalar2=0.5, op0=OP.max, op1=OP.mult)
            inter = tmp(TA)
            nc.vector.tensor_tensor(out=inter[:], in0=_v(iwih[:], 0, [[1, TA]]),
                                    in1=_v(iwih[:], TA, [[1, TA]]), op=OP.mult)
            # union side branch on gpsimd, in parallel with the inter chain
            areaA = tmp(TA)
            nc.gpsimd.tensor_tensor(out=areaA[:], in0=_v(wf[:], 0, [[1, TA]]),
                                    in1=_v(wf[:], TA, [[1, TA]]), op=OP.mult)
            u1 = tmp(TA)
            nc.gpsimd.tensor_tensor(out=_v(u1[:], 0, [[A, T], [1, A]]),
                                    in0=_v(areaA[:], 0, [[A, T], [1, A]]),
                                    in1=_v(fp[:], O["TAREA"], [[1, T], [0, A]]),
                                    op=OP.add)
            u2 = tmp(TA)
            nc.vector.tensor_tensor(out=u2[:], in0=u1[:], in1=inter[:],
                                    op=OP.subtract)
            rcp = tmp(TA)
            nc.vector.reciprocal(out=rcp[:], in_=u2[:])
            iou = tmp(TA)
            nc.vector.tensor_tensor(out=iou[:], in0=inter[:], in1=rcp[:],
                                    op=OP.mult)

            # first-argmax over a -> fmask (exact float equality + tiebreak)
            rmax = tmp(T)
            nc.vector.tensor_reduce(out=rmax[:],
                                    in_=_v(iou[:], 0, [[A, T], [1, A]]),
                                    axis=AX.X, op=OP.max)
            eq = tmp(TA)
            nc.vector.tensor_tensor(out=_v(eq[:], 0, [[A, T], [1, A]]),
                                    in0=_v(iou[:], 0, [[A, T], [1, A]]),
                                    in1=_v(rmax[:], 0, [[1, T], [0, A]]),
                                    op=OP.is_equal)
            # WOBJ = (A - a) * obj: padding cells give fval = 0 -> fmask all-1
            # there, which is harmless: u/uu/ce/box are all zero at padding.
            fval = tmp(TA)
            nc.vector.tensor_tensor(out=_v(fval[:], 0, [[A, T], [1, A]]),
                                    in0=_v(eq[:], 0, [[A, T], [1, A]]),
                                    in1=_v(fp[:], O["WCONST"], [[A, T], [1, A]]),
                                    op=OP.mult)
            m2 = tmp(T)
            nc.vector.tensor_reduce(out=m2[:],
                                    in_=_v(fval[:], 0, [[A, T], [1, A]]),
                                    axis=AX.X, op=OP.max)
            fmask = tmp(TA)
            nc.vector.tensor_tensor(out=_v(fmask[:], 0, [[A, T], [1, A]]),
                                    in0=_v(fval[:], 0, [[A, T], [1, A]]),
                                    in1=_v(m2[:], 0, [[1, T], [0, A]]),
                                    op=OP.is_equal)
            # cls path: se = sum_j e (bf16), lse bit trick + ce on gpsimd
            se = tmp(TA, bf16)
            with nc.allow_low_precision(reason="lse bit-trick needs bf16 bits; "
                                        "2e-2 loss tolerance"):
                nc.vector.tensor_reduce(
                    out=_v(se[:], 0, [[1, TA]]),
                    in_=_v(e[:], 0, [[NCLS, TA], [1, NCLS]]),
                    axis=AX.X, op=OP.add)
            lgf = tmp(TA)
            nc.gpsimd.tensor_copy(out=lgf[:], in_=se[:].bitcast(i16))
            lse = tmp(TA)
            nc.gpsimd.tensor_scalar(out=lse[:], in0=lgf[:],
                                    scalar1=LN2 / (1 << 7),
                                    scalar2=-LOG_BIAS * LN2,
                                    op0=OP.mult, op1=OP.add)
            nc.gpsimd.tensor_tensor(out=_v(SRC[:], 6 * TA, [[A, T], [1, A]]),
                                    in0=_v(lse[:], 0, [[A, T], [1, A]]),
                                    in1=_v(fp[:], O["SAUX"], [[A, T], [1, A]]),
                                    op=OP.subtract)

            # best-anchor selection of all seven quantities in one mul+reduce
            selm = pool.tile([P, 7 * TA], f32, name="selm")
            nc.vector.tensor_tensor(out=_v(selm[:], 0, [[TA, 7], [1, TA]]),
                                    in0=_v(SRC[:], 0, [[TA, 7], [1, TA]]),
                                    in1=_v(fmask[:], 0, [[0, 7], [1, TA]]),
                                    op=OP.mult)
            FIN = pool.tile([P, 7 * T], f32, name="FIN")
            nc.vector.tensor_reduce(out=_v(FIN[:], 0, [[T, 7], [1, T]]),
                                    in_=_v(selm[:], 0, [[TA, 7], [A, T], [1, A]]),
                                    axis=AX.X, op=OP.add)

            # box: FIN q0..3 -> oscl4*(sel - aux4)^2, written back into q0..3
            d4 = tmp(4 * T)
            nc.vector.tensor_tensor(out=d4[:], in0=_v(FIN[:], 0, [[1, 4 * T]]),
                                    in1=_v(fp[:], O["AUX4"], [[1, 4 * T]]),
                                    op=OP.subtract)
            d4m = tmp(4 * T)
            nc.vector.tensor_tensor(out=d4m[:], in0=d4[:],
                                    in1=_v(fp[:], O["OSCL4"], [[1, 4 * T]]),
                                    op=OP.mult)
            nc.vector.tensor_tensor(out=_v(FIN[:], 0, [[1, 4 * T]]),
                                    in0=d4[:], in1=d4m[:], op=OP.mult)

            # one grouped reduce -> partials cols 0..6
            nc.vector.tensor_reduce(out=_v(partials[:], 0, [[1, 7]]),
                                    in_=_v(FIN[:], 0, [[T, 7], [1, T]]),
                                    axis=AX.X, op=OP.add)

            nc.sync.dma_start(out=partials_d[:], in_=partials[:])

    if split:
        _split_multi_waits(nc)
    return nc


# -------------------------------------------------------------- shard builder
def _make_in_maps(out, gt_boxes, anchor_np, gt_classes_np, num_box_np, T):
    obj, xo, yo, tw, th, cls_t = _build_target_np(gt_boxes, gt_classes_np,
                                                  num_box_np)
    SLOTS = P * T
    TA = T * A
    out_r = out.reshape(B, A, 25, HWC)
    sqa = np.sqrt(anchor_np)                       # [A, 2]

    in_maps = []
    for c in range(CORES):
        sl = slice(c * BC, (c + 1) * BC)
        ob = obj[sl]                               # [BC, HWC]
        bloc, hwloc = np.nonzero(ob > 0)
        K = len(bloc)
        assert K <= SLOTS

        def place(vals):
            buf = np.zeros(SLOTS, dtype=np.float32)
            buf[:K] = vals
            return buf.reshape(P, T)

        objv = place(np.ones(K, dtype=np.float32))
        xov = place(xo[sl][bloc, hwloc])
        yov = place(yo[sl][bloc, hwloc])
        twv = place(tw[sl][bloc, hwloc])
        thv = place(th[sl][bloc, hwloc])

        # occupied-cell prediction channels [K, A, 25]
        colsb = np.zeros((SLOTS, A, 25), dtype=np.float32)
        if K:
            colsb[:K] = out_r[sl].transpose(0, 3, 1, 2)[bloc, hwloc]
        # cols_xw (t, ch{x,y,conf,w,h}, a)
        sel = colsb[:, :, [21, 22, 20, 23, 24]]            # (t, a, ch)
        cols_xw = np.ascontiguousarray(
            sel.reshape(P, T, A, 5).transpose(0, 1, 3, 2)).reshape(P, 25 * T)
        logits = np.ascontiguousarray(
            colsb[:, :, :20]).reshape(P, 100 * T)          # (t, a, j)

        # target-class logit per (t, a); padding slots get the exact device
        # lse of se=20.0 (logits 0 -> e all 1) so padded ce comes out 0
        clsv = place(cls_t[sl][bloc, hwloc].astype(np.float32)).astype(np.int64)
        s_aux = np.take_along_axis(
            colsb[:, :, :20].reshape(SLOTS, A, 20),
            clsv.reshape(SLOTS, 1, 1).repeat(A, axis=1), axis=2
        )[:, :, 0].astype(np.float32)
        i16pad = float(_bf16(np.float32(20.0)).view(np.int16))
        lse_pad = np.float32(np.float32(i16pad * np.float32(LN2 / (1 << 7)))
                             + np.float32(-LOG_BIAS * LN2))
        s_aux[K:] = lse_pad
        s_aux = s_aux.reshape(P, TA)

        # xi-space target box edges (d{x,y}, t): center 2o-1, half-width t_wh
        cxv = 2.0 * xov - 1.0
        cyv = 2.0 * yov - 1.0
        b1 = np.stack([cxv - twv, cyv - thv], axis=1).reshape(P, 2 * T)
        b2 = np.stack([cxv + twv, cyv + thv], axis=1).reshape(P, 2 * T)
        tarea = (twv * thv).reshape(P, T)

        # (A - a) * obj in (t, a) layout: padding cells get all-zero fval
        wconst = np.ascontiguousarray(
            (A - np.arange(A, dtype=np.float32))[None, None, :]
            * objv[:, :, None]).reshape(P, 5 * T)
        sqav = np.broadcast_to(sqa.reshape(1, 10), (P, 10))

        # AUX4 / OSCL4 in q-order (w, h, x, y)
        aux4 = np.stack([np.sqrt(twv), np.sqrt(thv), cxv, cyv],
                        axis=1).reshape(P, 4 * T)
        oscl4 = np.stack([objv, objv, 0.25 * objv, 0.25 * objv],
                         axis=1).reshape(P, 4 * T)

        fpack = np.concatenate(
            [cols_xw, b1, b2, tarea, wconst, sqav,
             s_aux, aux4, oscl4, objv.reshape(P, T)], axis=1)

        # dense conf channels: [BC, A, HWC] -> [P, 1280] bf16
        confd = out_r[sl][:, :, 20, :].reshape(P, -1)

        in_maps.append({
            "fpack": np.ascontiguousarray(fpack, dtype=np.float32),
            "lgpack": _bf16(logits),
            "confd": _bf16(confd),
        })
    return in_maps


# ---------------------------------------------------------------- entry point
def kernel(out, gt_boxes, anchor, gt_classes, num_box):
    from concourse.bass_utils import run_bass_kernel_spmd

    out = np.ascontiguousarray(np.asarray(out, dtype=np.float32))
    gt_boxes = np.asarray(gt_boxes, dtype=np.float32)
    anchor_np = np.asarray(anchor, dtype=np.float32)
    gt_classes_np = np.asarray(gt_classes)
    num_box_np = np.asarray(num_box)

    # per-core occupied-cell counts decide the compiled tile factor T
    obj = _build_target_np(gt_boxes, gt_classes_np, num_box_np)[0]
    ks = [int((obj[c * BC:(c + 1) * BC] > 0).sum()) for c in range(CORES)]
    maxk = max(ks)
    T = max(1, -(-maxk // P))
    assert maxk <= 13 * P and T <= 13

    in_maps = _make_in_maps(out, gt_boxes, anchor_np, gt_classes_np,
                            num_box_np, T)

    import os
    key = f"nc{T}"
    if key not in _CACHE:
        _CACHE[key] = _build_nc(T)
    trace = os.environ.get("KERNEL_TRACE", "0") == "1"
    res = run_bass_kernel_spmd(_CACHE[key], in_maps, core_ids=list(range(CORES)),
                               trace=trace)
    if trace:
        print(f"HW exec time: {res.exec_time_ns} ns  (mean {res.mean_exec_time_ns})")

    cols = np.zeros(12, dtype=np.float64)
    for c in range(CORES):
        cols += res.results[c]["partials"].astype(np.float64).sum(axis=0)
    K = float(sum(ks))
    box_loss = np.float32(LAM_COORD / B * (cols[0] + cols[1] + cols[2] + cols[3]))
    conf_loss = np.float32(LAM_OBJ / B * 0.25 * (cols[5] - 2.0 * cols[4] + K))
    nob_c = 0.25 * (cols[5] + 2.0 * cols[4] + K)
    dense = 0.25 * (float(B * A * HWC) + 2.0 * cols[7] + cols[8])
    noobj_loss = np.float32(LAM_NOOBJ / B * (dense - nob_c))
    cls_loss = np.float32(LAM_CLS / B * cols[6])
    return (box_loss, conf_loss, noobj_loss, cls_loss)


# revision 38
# speedup vs baseline: 1.0558x; 1.0004x over previous
"""Trainium2 Bass kernel for nn_Loss_65781719105930 (YOLO-style detection loss).

Strategy (pure data parallelism, 8 cores, 32 images each):
  host:   replicate the reference's target-build scatter (small int64 inputs),
          compact occupied cells, pre-pack aux tables + prediction columns into
          three contiguous DMA payloads; gather the target-class logit per
          (cell, anchor) host-side.
  device: dense pass over the 5 conf channels (sum of sigmoid^2), plus IoU /
          first-argmax / best-anchor-select / cross-entropy on compacted tiles.

Numeric tricks that keep the scalar engine on ONE activation-table set
(exp_and_others = {tanh, exp, square}):
  sigmoid(x)   = (1 + tanh(x/2)) / 2      -> work in xi = 2x-1 coords; the
                                             0.5 factors fold into host consts
  sqrt(exp(x)*anchor) = exp(x/2)*sqrt(anchor)
  ln(x)        ~ bitcast_i16(bf16 x) * ln2/2^7 - 126.94269504*ln2
                 (mean-centered log2 bit trick; loss tolerance is 2e-2 rel)

The grid offset cancels algebraically in both the IoU and the box loss, so it
never appears on device.

Device program layout: SRC [P, 7*T*A] holds quantities q = (w, h, x, y, u, uu,
ce) per (cell t, anchor a); one mul by (fmask*obj) + one reduce selects the
best anchor for all seven; one final grouped reduce produces all loss partial
sums at once.
"""
import numpy as np

# ---------------------------------------------------------------- constants
NCLS = 20
H = W = 32
HWC = H * W            # 1024 cells/image
A = 5
M = 50
B = 256
CORES = 8
BC = B // CORES        # 32 images per core
CH = A * (5 + NCLS)    # 125 channels
P = 128
LAM_COORD, LAM_OBJ, LAM_NOOBJ, LAM_CLS = 5.0, 1.0, 0.5, 1.0

LN2 = float(np.log(2.0))
LOG_BIAS = 126.94269504   # mean-centering constant for the log2 bit trick

_CACHE = {}


def _bf16(x):
    """float32 ndarray -> ml_dtypes.bfloat16 (RNE)."""
    import ml_dtypes
    return np.asarray(x, dtype=np.float32).astype(ml_dtypes.bfloat16)


# ---------------------------------------------------------------- host prep
def _build_target_np(gt_boxes, gt_classes, num_box):
    """Numpy replication of reference.build_target (last object wins, first-max
    class argmax). Returns per-cell [B, HWC] arrays."""
    Bn = gt_boxes.shape[0]
    valid = np.arange(M)[None, :] < num_box[:, None]
    x = gt_boxes[..., 0].astype(np.float32) * H
    y = gt_boxes[..., 1].astype(np.float32) * H
    gx = np.floor(x).astype(np.int64)
    gy = np.floor(y).astype(np.int64)
    flat = np.where(valid, gy * W + gx, HWC)
    bi = np.broadcast_to(np.arange(Bn)[:, None], (Bn, M))

    vals = np.stack([np.ones_like(x), x - gx, y - gy,
                     gt_boxes[..., 2].astype(np.float32) * H,
                     gt_boxes[..., 3].astype(np.float32) * H], axis=-1)
    tgt_box = np.zeros((Bn, HWC + 1, 5), dtype=np.float32)
    tgt_box[bi, flat] = vals
    tgt_cls = np.zeros((Bn, HWC + 1, NCLS), dtype=np.float32)
    tgt_cls[bi, flat, gt_classes.astype(np.int64)] = 1.0

    tgt_box = tgt_box[:, :HWC]
    obj = tgt_box[..., 0]
    cls_t = np.argmax(tgt_cls[:, :HWC], axis=-1).astype(np.int64)
    return obj, tgt_box[..., 1], tgt_box[..., 2], tgt_box[..., 3], tgt_box[..., 4], cls_t


def _split_multi_waits(nc):
    """This container's walrus accepts only ONE sem-wait per instruction; hoist
    extra waits onto standalone NoOps."""
    import concourse.mybir as mybir
    import bass_rust
    n = 0
    for fn in nc.m.functions:
        for blk in fn.blocks:
            new = []
            for ins in blk.instructions:
                si = ins.sync_info
                waits = list(si.on_wait) if si is not None else []
                if len(waits) > 1:
                    for w in waits[:-1]:
                        nop = mybir.InstNoOp(name=f"{ins.name}-w{n}")
                        nop.engine = ins.engine
                        nop.sync_info = bass_rust.SyncInfo(on_wait=[w], on_update=[])
                        new.append(nop)
                        n += 1
                    si.on_wait = [waits[-1]]
                    ins.sync_info = si
                new.append(ins)
            blk.instructions = new
    return n


def _offsets(T):
    """fpack free-dim offsets. cols_xw channel order is (x, y, conf, w, h)."""
    o = {}
    o["XW"] = 0
    o["B1"] = 25 * T
    o["B2"] = o["B1"] + 2 * T
    o["TAREA"] = o["B2"] + 2 * T
    o["WCONST"] = o["TAREA"] + T
    o["SQA"] = o["WCONST"] + 5 * T
    o["SAUX"] = o["SQA"] + 10
    o["AUX4"] = o["SAUX"] + 5 * T
    o["OSCL4"] = o["AUX4"] + 4 * T
    o["OBJ"] = o["OSCL4"] + 4 * T
    o["NF"] = o["OBJ"] + T
    return o


# ---------------------------------------------------------------- bass build
def _build_nc(T, split=True):
    """Build the per-core kernel for T cell-blocks per partition (P*T slots).

    fpack [P, NF] f32:
      cols_xw (t, ch{x,y,conf,w,h}, a)  25T
      B1, B2  (d{x,y}, t)               2T each   xi-space target box edges
      TAREA   (t)                       T         tw*th (physical, cell units)
      WCONST  (a)                       5         A - a (first-argmax tiebreak)
      SQA     (d{w,h}, a)               10        sqrt(anchor)
      S_AUX   (t, a)                    5T        target-class logit
      AUX4    (q{w,h,x,y}, t)           4T        (sqrt tw, sqrt th, 2xo-1, 2yo-1)
      OSCL4   (q, t)                    4T        obj * {1,1,.25,.25}
      OBJ     (t)                       T
    lgpack bf16 [P, 100T]: logits (t, a, j)
    confd  bf16 [P, 1280]: all conf channels (dense noobj pass)
    partials out [P, 12]:
      0..3 box (w,h,x,y) sq-diff sums, 4 sum obj*u, 5 sum obj*u^2,
      6 sum obj*ce, 7 dense sum ud, 8 dense sum ud^2   (u/ud = tanh(conf/2))
    """
    import concourse.bass as bass
    import concourse.mybir as mybir
    import concourse.tile as tile

    f32 = mybir.dt.float32
    bf16 = mybir.dt.bfloat16
    i16 = mybir.dt.int16
    AF = mybir.ActivationFunctionType
    OP = mybir.AluOpType
    AX = mybir.AxisListType

    TA = T * A
    TA2 = TA * 2
    O = _offsets(T)
    NF = O["NF"]
    DF = BC * A * HWC // P   # 1280 dense conf elements per partition

    def _v(ap, off, dims):
        """Sub-view of a tile AP: keep its partition dim, replace free dims."""
        return bass.AP(tensor=ap.tensor, offset=ap.offset + off,
                       ap=[list(ap.ap[0])] + dims)

    nc = bass.Bass("TRN2")
    fpack_d = nc.declare_dram_parameter("fpack", [P, NF], f32, isOutput=False)
    lgpack_d = nc.declare_dram_parameter("lgpack", [P, 100 * T], bf16, isOutput=False)
    confd_d = nc.declare_dram_parameter("confd", [P, DF], bf16, isOutput=False)
    partials_d = nc.declare_dram_parameter("partials", [P, 12], f32, isOutput=True)

    with tile.TileContext(nc) as tc:
        with tc.tile_pool(name="sb", bufs=1) as pool:
            # ---------------- input DMAs, priority order, on sync HWDGE
            fp = pool.tile([P, NF], f32, name="fp")
            nc.sync.dma_start(out=fp[:], in_=fpack_d[:])
            lg_in = pool.tile([P, 100 * T], bf16, name="lg_in")
            nc.sync.dma_start(out=lg_in[:], in_=lgpack_d[:])
            confd = pool.tile([P, DF], bf16, name="confd")
            nc.sync.dma_start(out=confd[:], in_=confd_d[:])

            partials = pool.tile([P, 12], f32, name="partials")

            # SRC: (q, t, a) with q in {w, h, x, y, u, uu, ce}
            SRC = pool.tile([P, 7 * TA], f32, name="SRC")

            # ---------------- scalar engine program (one act-table set)
            # EW = exp(chan{w,h}/2): chans 3,4 of cols_xw -> (t, a, d)
            EW = pool.tile([P, TA2], f32, name="EW")
            nc.scalar.activation(
                _v(EW[:], 0, [[1, 2], [2 * A, T], [2, A]]),
                _v(fp[:], O["XW"] + 15, [[5, 2], [25, T], [1, A]]),
                AF.Exp, scale=0.5)
            # x, y, u = tanh(chan{x,y,conf}/2) -> SRC q2, q3, q4
            nc.scalar.activation(
                _v(SRC[:], 2 * TA, [[TA, 3], [A, T], [1, A]]),
                _v(fp[:], O["XW"], [[5, 3], [25, T], [1, A]]),
                AF.Tanh, scale=0.5)
            # e = exp(logits), bf16, (t, a, j)
            e = pool.tile([P, 100 * T], bf16, name="e")
            nc.scalar.activation(
                _v(e[:], 0, [[100, T], [NCLS, A], [1, NCLS]]),
                _v(lg_in[:], 0, [[100, T], [NCLS, A], [1, NCLS]]),
                AF.Exp)
            # dense noobj pass: ud = tanh(c/2); sum sigma^2 = 0.25*(N + 2*sum ud
            # + sum ud^2)
            # uu = u^2 -> SRC q5 (square is in the same act-table set)
            nc.scalar.activation(_v(SRC[:], 5 * TA, [[1, TA]]),
                                 _v(SRC[:], 4 * TA, [[1, TA]]), AF.Square)
            UD = pool.tile([P, DF], f32, name="UD")
            nc.scalar.activation(UD[:], confd[:], AF.Tanh, scale=0.5,
                                 accum_out=_v(partials[:], 7, [[1, 1]]))
            SQD = pool.tile([P, DF], f32, name="SQD")
            nc.scalar.activation(SQD[:], UD[:], AF.Square,
                                 accum_out=_v(partials[:], 8, [[1, 1]]))

            # ---------------- vector engine program
            tcnt = [0]

            def tmp(n, dtype=f32):
                tcnt[0] += 1
                return pool.tile([P, n], dtype, name=f"t{tcnt[0]}")

            # sh = EW * sqrt(anchor) = sqrt(pred_wh) -> SRC q0, q1
            SH = _v(SRC[:], 0, [[TA, 2], [A, T], [1, A]])
            nc.vector.tensor_tensor(
                out=SH,
                in0=_v(EW[:], 0, [[1, 2], [2 * A, T], [2, A]]),
                in1=_v(fp[:], O["SQA"], [[1, 2], [0, T], [2, A]]),
                op=OP.mult)
            # wfull = sh*sh = pred_wh (xi-space half-width), (d, t, a)
            wf = tmp(TA2)
            SH2 = _v(SRC[:], 0, [[TA, 2], [1, TA]])
            WF = _v(wf[:], 0, [[TA, 2], [1, TA]])
            nc.vector.tensor_tensor(out=WF, in0=SH2, in1=SH2, op=OP.mult)

            # IoU in xi coords. XY = SRC q2, q3 as (d, t, a)
            XY = _v(SRC[:], 2 * TA, [[TA, 2], [1, TA]])
            lo = tmp(TA2)
            nc.vector.tensor_tensor(out=lo[:], in0=XY, in1=WF, op=OP.subtract)
            hi = tmp(TA2)
            nc.vector.tensor_tensor(out=hi[:], in0=XY, in1=WF, op=OP.add)
            B1v = _v(fp[:], O["B1"], [[T, 2], [1, T], [0, A]])
            B2v = _v(fp[:], O["B2"], [[T, 2], [1, T], [0, A]])
            LOv = _v(lo[:], 0, [[TA, 2], [A, T], [1, A]])
            HIv = _v(hi[:], 0, [[TA, 2], [A, T], [1, A]])
            t1 = tmp(TA2)
            nc.vector.tensor_tensor(out=_v(t1[:], 0, [[TA, 2], [A, T], [1, A]]),
                                    in0=HIv, in1=B2v, op=OP.min)
            t2 = tmp(TA2)
            nc.vector.tensor_tensor(out=_v(t2[:], 0, [[TA, 2], [A, T], [1, A]]),
                                    in0=LOv, in1=B1v, op=OP.max)
            t3 = tmp(TA2)
            nc.vector.tensor_tensor(out=t3[:], in0=t1[:], in1=t2[:], op=OP.subtract)
            # iw = max(t3, 0) * 0.5 -> physical overlap widths (d, t, a)
            iwih = tmp(TA2)
            nc.vector.tensor_scalar(out=iwih[:], in0=t3[:], scalar1=0.0,
                                    scalar2=0.5, op0=OP.max, op1=OP.mult)
            inter = tmp(TA)
            nc.vector.tensor_tensor(out=inter[:], in0=_v(iwih[:], 0, [[1, TA]]),
                                    in1=_v(iwih[:], TA, [[1, TA]]), op=OP.mult)
            # union side branch on gpsimd, in parallel with the inter chain
            areaA = tmp(TA)
            nc.gpsimd.tensor_tensor(out=areaA[:], in0=_v(wf[:], 0, [[1, TA]]),
                                    in1=_v(wf[:], TA, [[1, TA]]), op=OP.mult)
            u1 = tmp(TA)
            nc.gpsimd.tensor_tensor(out=_v(u1[:], 0, [[A, T], [1, A]]),
                                    in0=_v(areaA[:], 0, [[A, T], [1, A]]),
                                    in1=_v(fp[:], O["TAREA"], [[1, T], [0, A]]),
                                    op=OP.add)
            u2 = tmp(TA)
            nc.vector.tensor_tensor(out=u2[:], in0=u1[:], in1=inter[:],
                                    op=OP.subtract)
            rcp = tmp(TA)
            nc.vector.reciprocal(out=rcp[:], in_=u2[:])
            iou = tmp(TA)
            nc.vector.tensor_tensor(out=iou[:], in0=inter[:], in1=rcp[:],
                                    op=OP.mult)

            # first-argmax over a -> fmask (exact float equality + tiebreak)
            rmax = tmp(T)
            nc.vector.tensor_reduce(out=rmax[:],
                                    in_=_v(iou[:], 0, [[A, T], [1, A]]),
                                    axis=AX.X, op=OP.max)
            eq = tmp(TA)
            nc.vector.tensor_tensor(out=_v(eq[:], 0, [[A, T], [1, A]]),
                                    in0=_v(iou[:], 0, [[A, T], [1, A]]),
                                    in1=_v(rmax[:], 0, [[1, T], [0, A]]),
                                    op=OP.is_equal)
            # WOBJ = (A - a) * obj: padding cells give fval = 0 -> fmask all-1
            # there, which is harmless: u/uu/ce/box are all zero at padding.
            fval = tmp(TA)
            nc.vector.tensor_tensor(out=_v(fval[:], 0, [[A, T], [1, A]]),
                                    in0=_v(eq[:], 0, [[A, T], [1, A]]),
                                    in1=_v(fp[:], O["WCONST"], [[A, T], [1, A]]),
                                    op=OP.mult)
            m2 = tmp(T)
            nc.vector.tensor_reduce(out=m2[:],
                                    in_=_v(fval[:], 0, [[A, T], [1, A]]),
                                    axis=AX.X, op=OP.max)
            fmask = tmp(TA)
            nc.vector.tensor_tensor(out=_v(fmask[:], 0, [[A, T], [1, A]]),
                                    in0=_v(fval[:], 0, [[A, T], [1, A]]),
                                    in1=_v(m2[:], 0, [[1, T], [0, A]]),
                                    op=OP.is_equal)
            # cls path: se = sum_j e (bf16), lse bit trick + ce on gpsimd
            se = tmp(TA, bf16)
            with nc.allow_low_precision(reason="lse bit-trick needs bf16 bits; "
                                        "2e-2 loss tolerance"):
                nc.vector.tensor_reduce(
                    out=_v(se[:], 0, [[1, TA]]),
                    in_=_v(e[:], 0, [[NCLS, TA], [1, NCLS]]),
                    axis=AX.X, op=OP.add)
            lgf = tmp(TA)
            nc.gpsimd.tensor_copy(out=lgf[:], in_=se[:].bitcast(i16))
            lse = tmp(TA)
            nc.gpsimd.tensor_scalar(out=lse[:], in0=lgf[:],
                                    scalar1=LN2 / (1 << 7),
                                    scalar2=-LOG_BIAS * LN2,
                                    op0=OP.mult, op1=OP.add)
            nc.gpsimd.tensor_tensor(out=_v(SRC[:], 6 * TA, [[A, T], [1, A]]),
                                    in0=_v(lse[:], 0, [[A, T], [1, A]]),
                                    in1=_v(fp[:], O["SAUX"], [[A, T], [1, A]]),
                                    op=OP.subtract)

            # best-anchor selection of all seven quantities in one mul+reduce
            selm = pool.tile([P, 7 * TA], f32, name="selm")
            nc.vector.tensor_tensor(out=_v(selm[:], 0, [[TA, 7], [1, TA]]),
                                    in0=_v(SRC[:], 0, [[TA, 7], [1, TA]]),
                                    in1=_v(fmask[:], 0, [[0, 7], [1, TA]]),
                                    op=OP.mult)
            FIN = pool.tile([P, 7 * T], f32, name="FIN")
            nc.vector.tensor_reduce(out=_v(FIN[:], 0, [[T, 7], [1, T]]),
                                    in_=_v(selm[:], 0, [[TA, 7], [A, T], [1, A]]),
                                    axis=AX.X, op=OP.add)

            # box: FIN q0..3 -> oscl4*(sel - aux4)^2, written back into q0..3
            d4 = tmp(4 * T)
            nc.vector.tensor_tensor(out=d4[:], in0=_v(FIN[:], 0, [[1, 4 * T]]),
                                    in1=_v(fp[:], O["AUX4"], [[1, 4 * T]]),
                                    op=OP.subtract)
            d4m = tmp(4 * T)
            nc.vector.tensor_tensor(out=d4m[:], in0=d4[:],
                                    in1=_v(fp[:], O["OSCL4"], [[1, 4 * T]]),
                                    op=OP.mult)
            nc.vector.tensor_tensor(out=_v(FIN[:], 0, [[1, 4 * T]]),
                                    in0=d4[:], in1=d4m[:], op=OP.mult)

            # one grouped reduce -> partials cols 0..6
            nc.vector.tensor_reduce(out=_v(partials[:], 0, [[1, 7]]),
                                    in_=_v(FIN[:], 0, [[T, 7], [1, T]]),
                                    axis=AX.X, op=OP.add)

            nc.sync.dma_start(out=partials_d[:], in_=partials[:])

    if split:
        _split_multi_waits(nc)
    _strip_unused_const_memsets(nc)
    return nc


def _strip_unused_const_memsets(nc):
    """Drop the Bass-prologue memsets for const APs this kernel never reads
    (only const-float32-0.0 is used, as the activation bias)."""
    drop = ("const-float32-1.0", "const-bfloat16-1.0", "const-uint8-127")
    for fn in nc.m.functions:
        for blk in fn.blocks:
            keep = []
            for ins in blk.instructions:
                outs = getattr(ins, "outs", None) or []
                names = {getattr(getattr(o, "tensor", None), "name", "")
                         for o in outs}
                if type(ins).__name__ == "InstMemSet" and names & set(drop):
                    continue
                keep.append(ins)
            blk.instructions = keep


# -------------------------------------------------------------- shard builder
def _make_in_maps(out, gt_boxes, anchor_np, gt_classes_np, num_box_np, T):
    obj, xo, yo, tw, th, cls_t = _build_target_np(gt_boxes, gt_classes_np,
                                                  num_box_np)
    SLOTS = P * T
    TA = T * A
    out_r = out.reshape(B, A, 25, HWC)
    sqa = np.sqrt(anchor_np)                       # [A, 2]

    in_maps = []
    for c in range(CORES):
        sl = slice(c * BC, (c + 1) * BC)
        ob = obj[sl]                               # [BC, HWC]
        bloc, hwloc = np.nonzero(ob > 0)
        K = len(bloc)
        assert K <= SLOTS

        def place(vals):
            buf = np.zeros(SLOTS, dtype=np.float32)
            buf[:K] = vals
            return buf.reshape(P, T)

        objv = place(np.ones(K, dtype=np.float32))
        xov = place(xo[sl][bloc, hwloc])
        yov = place(yo[sl][bloc, hwloc])
        twv = place(tw[sl][bloc, hwloc])
        thv = place(th[sl][bloc, hwloc])

        # occupied-cell prediction channels [K, A, 25]
        colsb = np.zeros((SLOTS, A, 25), dtype=np.float32)
        if K:
            colsb[:K] = out_r[sl].transpose(0, 3, 1, 2)[bloc, hwloc]
        # cols_xw (t, ch{x,y,conf,w,h}, a)
        sel = colsb[:, :, [21, 22, 20, 23, 24]]            # (t, a, ch)
        cols_xw = np.ascontiguousarray(
            sel.reshape(P, T, A, 5).transpose(0, 1, 3, 2)).reshape(P, 25 * T)
        logits = np.ascontiguousarray(
            colsb[:, :, :20]).reshape(P, 100 * T)          # (t, a, j)

        # target-class logit per (t, a); padding slots get the exact device
        # lse of se=20.0 (logits 0 -> e all 1) so padded ce comes out 0
        clsv = place(cls_t[sl][bloc, hwloc].astype(np.float32)).astype(np.int64)
        s_aux = np.take_along_axis(
            colsb[:, :, :20].reshape(SLOTS, A, 20),
            clsv.reshape(SLOTS, 1, 1).repeat(A, axis=1), axis=2
        )[:, :, 0].astype(np.float32)
        i16pad = float(_bf16(np.float32(20.0)).view(np.int16))
        lse_pad = np.float32(np.float32(i16pad * np.float32(LN2 / (1 << 7)))
                             + np.float32(-LOG_BIAS * LN2))
        s_aux[K:] = lse_pad
        s_aux = s_aux.reshape(P, TA)

        # xi-space target box edges (d{x,y}, t): center 2o-1, half-width t_wh
        cxv = 2.0 * xov - 1.0
        cyv = 2.0 * yov - 1.0
        b1 = np.stack([cxv - twv, cyv - thv], axis=1).reshape(P, 2 * T)
        b2 = np.stack([cxv + twv, cyv + thv], axis=1).reshape(P, 2 * T)
        tarea = (twv * thv).reshape(P, T)

        # (A - a) * obj in (t, a) layout: padding cells get all-zero fval
        wconst = np.ascontiguousarray(
            (A - np.arange(A, dtype=np.float32))[None, None, :]
            * objv[:, :, None]).reshape(P, 5 * T)
        sqav = np.broadcast_to(sqa.reshape(1, 10), (P, 10))

        # AUX4 / OSCL4 in q-order (w, h, x, y)
        aux4 = np.stack([np.sqrt(twv), np.sqrt(thv), cxv, cyv],
                        axis=1).reshape(P, 4 * T)
        oscl4 = np.stack([objv, objv, 0.25 * objv, 0.25 * objv],
                         axis=1).reshape(P, 4 * T)

        fpack = np.concatenate(
            [cols_xw, b1, b2, tarea, wconst, sqav,
             s_aux, aux4, oscl4, objv.reshape(P, T)], axis=1)

        # dense conf channels: [BC, A, HWC] -> [P, 1280] bf16
        confd = out_r[sl][:, :, 20, :].reshape(P, -1)

        in_maps.append({
            "fpack": np.ascontiguousarray(fpack, dtype=np.float32),
            "lgpack": _bf16(logits),
            "confd": _bf16(confd),
        })
    return in_maps


# ---------------------------------------------------------------- entry point
def kernel(out, gt_boxes, anchor, gt_classes, num_box):
    from concourse.bass_utils import run_bass_kernel_spmd

    out = np.ascontiguousarray(np.asarray(out, dtype=np.float32))
    gt_boxes = np.asarray(gt_boxes, dtype=np.float32)
    anchor_np = np.asarray(anchor, dtype=np.float32)
    gt_classes_np = np.asarray(gt_classes)
    num_box_np = np.asarray(num_box)

    # per-core occupied-cell counts decide the compiled tile factor T
    obj = _build_target_np(gt_boxes, gt_classes_np, num_box_np)[0]
    ks = [int((obj[c * BC:(c + 1) * BC] > 0).sum()) for c in range(CORES)]
    maxk = max(ks)
    T = max(1, -(-maxk // P))
    assert maxk <= 13 * P and T <= 13

    in_maps = _make_in_maps(out, gt_boxes, anchor_np, gt_classes_np,
                            num_box_np, T)

    import os
    key = f"nc{T}"
    if key not in _CACHE:
        _CACHE[key] = _build_nc(T)
    trace = os.environ.get("KERNEL_TRACE", "0") == "1"
    res = run_bass_kernel_spmd(_CACHE[key], in_maps, core_ids=list(range(CORES)),
                               trace=trace)
    if trace:
        print(f"HW exec time: {res.exec_time_ns} ns  (mean {res.mean_exec_time_ns})")

    cols = np.zeros(12, dtype=np.float64)
    for c in range(CORES):
        cols += res.results[c]["partials"].astype(np.float64).sum(axis=0)
    K = float(sum(ks))
    box_loss = np.float32(LAM_COORD / B * (cols[0] + cols[1] + cols[2] + cols[3]))
    conf_loss = np.float32(LAM_OBJ / B * 0.25 * (cols[5] - 2.0 * cols[4] + K))
    nob_c = 0.25 * (cols[5] + 2.0 * cols[4] + K)
    dense = 0.25 * (float(B * A * HWC) + 2.0 * cols[7] + cols[8])
    noobj_loss = np.float32(LAM_NOOBJ / B * (dense - nob_c))
    cls_loss = np.float32(LAM_CLS / B * cols[6])
    return (box_loss, conf_loss, noobj_loss, cls_loss)


# revision 39
# speedup vs baseline: 1.0748x; 1.0180x over previous
"""Trainium2 Bass kernel for nn_Loss_65781719105930 (YOLO-style detection loss).

Strategy (pure data parallelism, 8 cores, 32 images each):
  host:   replicate the reference's target-build scatter (small int64 inputs),
          compact occupied cells, pre-pack aux tables + prediction columns into
          three contiguous DMA payloads; gather the target-class logit per
          (cell, anchor) host-side.
  device: dense pass over the 5 conf channels (sum of sigmoid^2), plus IoU /
          first-argmax / best-anchor-select / cross-entropy on compacted tiles.

Numeric tricks that keep the scalar engine on ONE activation-table set
(exp_and_others = {tanh, exp, square}):
  sigmoid(x)   = (1 + tanh(x/2)) / 2      -> work in xi = 2x-1 coords; the
                                             0.5 factors fold into host consts
  sqrt(exp(x)*anchor) = exp(x/2)*sqrt(anchor)
  ln(x)        ~ bitcast_i16(bf16 x) * ln2/2^7 - 126.94269504*ln2
                 (mean-centered log2 bit trick; loss tolerance is 2e-2 rel)

The grid offset cancels algebraically in both the IoU and the box loss, so it
never appears on device.

Device program layout: SRC [P, 7*T*A] holds quantities q = (w, h, x, y, u, uu,
ce) per (cell t, anchor a); one mul by (fmask*obj) + one reduce selects the
best anchor for all seven; one final grouped reduce produces all loss partial
sums at once.
"""
import numpy as np

# ---------------------------------------------------------------- constants
NCLS = 20
H = W = 32
HWC = H * W            # 1024 cells/image
A = 5
M = 50
B = 256
CORES = 8
BC = B // CORES        # 32 images per core
CH = A * (5 + NCLS)    # 125 channels
P = 128
LAM_COORD, LAM_OBJ, LAM_NOOBJ, LAM_CLS = 5.0, 1.0, 0.5, 1.0

LN2 = float(np.log(2.0))
LOG_BIAS = 126.94269504   # mean-centering constant for the log2 bit trick

_CACHE = {}


def _bf16(x):
    """float32 ndarray -> ml_dtypes.bfloat16 (RNE)."""
    import ml_dtypes
    return np.asarray(x, dtype=np.float32).astype(ml_dtypes.bfloat16)


# ---------------------------------------------------------------- host prep
def _build_target_np(gt_boxes, gt_classes, num_box):
    """Numpy replication of reference.build_target (last object wins, first-max
    class argmax). Returns per-cell [B, HWC] arrays."""
    Bn = gt_boxes.shape[0]
    valid = np.arange(M)[None, :] < num_box[:, None]
    x = gt_boxes[..., 0].astype(np.float32) * H
    y = gt_boxes[..., 1].astype(np.float32) * H
    gx = np.floor(x).astype(np.int64)
    gy = np.floor(y).astype(np.int64)
    flat = np.where(valid, gy * W + gx, HWC)
    bi = np.broadcast_to(np.arange(Bn)[:, None], (Bn, M))

    vals = np.stack([np.ones_like(x), x - gx, y - gy,
                     gt_boxes[..., 2].astype(np.float32) * H,
                     gt_boxes[..., 3].astype(np.float32) * H], axis=-1)
    tgt_box = np.zeros((Bn, HWC + 1, 5), dtype=np.float32)
    tgt_box[bi, flat] = vals
    tgt_cls = np.zeros((Bn, HWC + 1, NCLS), dtype=np.float32)
    tgt_cls[bi, flat, gt_classes.astype(np.int64)] = 1.0

    tgt_box = tgt_box[:, :HWC]
    obj = tgt_box[..., 0]
    cls_t = np.argmax(tgt_cls[:, :HWC], axis=-1).astype(np.int64)
    return obj, tgt_box[..., 1], tgt_box[..., 2], tgt_box[..., 3], tgt_box[..., 4], cls_t


def _split_multi_waits(nc):
    """This container's walrus accepts only ONE sem-wait per instruction; hoist
    extra waits onto standalone NoOps."""
    import concourse.mybir as mybir
    import bass_rust
    n = 0
    for fn in nc.m.functions:
        for blk in fn.blocks:
            new = []
            for ins in blk.instructions:
                si = ins.sync_info
                waits = list(si.on_wait) if si is not None else []
                if len(waits) > 1:
                    for w in waits[:-1]:
                        nop = mybir.InstNoOp(name=f"{ins.name}-w{n}")
                        nop.engine = ins.engine
                        nop.sync_info = bass_rust.SyncInfo(on_wait=[w], on_update=[])
                        new.append(nop)
                        n += 1
                    si.on_wait = [waits[-1]]
                    ins.sync_info = si
                new.append(ins)
            blk.instructions = new
    return n


def _offsets(T):
    """fpack free-dim offsets. cols_xw channel order is (x, y, conf, w, h)."""
    o = {}
    o["XW"] = 0
    o["B1"] = 25 * T
    o["B2"] = o["B1"] + 2 * T
    o["TAREA"] = o["B2"] + 2 * T
    o["WCONST"] = o["TAREA"] + T
    o["SQA"] = o["WCONST"] + 5 * T
    o["SAUX"] = o["SQA"] + 10
    o["AUX4"] = o["SAUX"] + 5 * T
    o["OSCL4"] = o["AUX4"] + 4 * T
    o["OBJ"] = o["OSCL4"] + 4 * T
    o["NF"] = o["OBJ"] + T
    return o


# ---------------------------------------------------------------- bass build
def _build_nc(T, split=True):
    """Build the per-core kernel for T cell-blocks per partition (P*T slots).

    fpack [P, NF] f32:
      cols_xw (t, ch{x,y,conf,w,h}, a)  25T
      B1, B2  (d{x,y}, t)               2T each   xi-space target box edges
      TAREA   (t)                       T         tw*th (physical, cell units)
      WCONST  (t, a)                    5T        (A - a)*obj (argmax tiebreak;
                                                  padding cells -> fval all 0)
      SQA     (d{w,h}, a)               10        sqrt(anchor)
      S_AUX   (t, a)                    5T        target-class logit
      AUX4    (q{w,h,x,y}, t)           4T        (sqrt tw, sqrt th, 2xo-1, 2yo-1)
      OSCL4   (q, t)                    4T        obj * {1,1,.25,.25}
      OBJ     (t)                       T
    lgpack bf16 [P, 100T]: logits (t, a, j)
    confd  bf16 [P, 1280]: all conf channels (dense noobj pass)
    partials out [P, 12]:
      0..3 box (w,h,x,y) sq-diff sums, 4 sum obj*u, 5 sum obj*u^2,
      6 sum obj*ce, 7 dense sum ud, 8 dense sum ud^2   (u/ud = tanh(conf/2))
    """
    import concourse.bass as bass
    import concourse.mybir as mybir
    import concourse.tile as tile

    f32 = mybir.dt.float32
    bf16 = mybir.dt.bfloat16
    i16 = mybir.dt.int16
    AF = mybir.ActivationFunctionType
    OP = mybir.AluOpType
    AX = mybir.AxisListType

    TA = T * A
    TA2 = TA * 2
    O = _offsets(T)
    NF = O["NF"]
    DF = BC * A * HWC // P   # 1280 dense conf elements per partition

    def _v(ap, off, dims):
        """Sub-view of a tile AP: keep its partition dim, replace free dims."""
        return bass.AP(tensor=ap.tensor, offset=ap.offset + off,
                       ap=[list(ap.ap[0])] + dims)

    nc = bass.Bass("TRN2")
    fpack_d = nc.declare_dram_parameter("fpack", [P, NF], f32, isOutput=False)
    lgpack_d = nc.declare_dram_parameter("lgpack", [P, 100 * T], bf16, isOutput=False)
    confd_d = nc.declare_dram_parameter("confd", [P, DF], bf16, isOutput=False)
    partials_d = nc.declare_dram_parameter("partials", [P, 12], f32, isOutput=True)

    with tile.TileContext(nc) as tc:
        with tc.tile_pool(name="sb", bufs=1) as pool:
            # ---------------- input DMAs, priority order, on sync HWDGE
            fp = pool.tile([P, NF], f32, name="fp")
            nc.sync.dma_start(out=fp[:], in_=fpack_d[:])
            lg_in = pool.tile([P, 100 * T], bf16, name="lg_in")
            nc.sync.dma_start(out=lg_in[:], in_=lgpack_d[:])
            confd = pool.tile([P, DF], bf16, name="confd")
            nc.sync.dma_start(out=confd[:], in_=confd_d[:])

            partials = pool.tile([P, 12], f32, name="partials")

            # SRC: (q, t, a) with q in {w, h, x, y, u, uu, ce}
            SRC = pool.tile([P, 7 * TA], f32, name="SRC")

            # ---------------- scalar engine program (one act-table set)
            # EW = exp(chan{w,h}/2): chans 3,4 of cols_xw -> (t, a, d)
            EW = pool.tile([P, TA2], f32, name="EW")
            nc.scalar.activation(
                _v(EW[:], 0, [[1, 2], [2 * A, T], [2, A]]),
                _v(fp[:], O["XW"] + 15, [[5, 2], [25, T], [1, A]]),
                AF.Exp, scale=0.5)
            # x, y, u = tanh(chan{x,y,conf}/2) -> SRC q2, q3, q4
            nc.scalar.activation(
                _v(SRC[:], 2 * TA, [[TA, 3], [A, T], [1, A]]),
                _v(fp[:], O["XW"], [[5, 3], [25, T], [1, A]]),
                AF.Tanh, scale=0.5)
            # e = exp(logits), bf16, (t, a, j)
            e = pool.tile([P, 100 * T], bf16, name="e")
            nc.scalar.activation(
                _v(e[:], 0, [[100, T], [NCLS, A], [1, NCLS]]),
                _v(lg_in[:], 0, [[100, T], [NCLS, A], [1, NCLS]]),
                AF.Exp)
            # dense noobj pass: ud = tanh(c/2); sum sigma^2 = 0.25*(N + 2*sum ud
            # + sum ud^2)
            # uu = u^2 -> SRC q5 (square is in the same act-table set)
            nc.scalar.activation(_v(SRC[:], 5 * TA, [[1, TA]]),
                                 _v(SRC[:], 4 * TA, [[1, TA]]), AF.Square)
            UD = pool.tile([P, DF], f32, name="UD")
            nc.scalar.activation(UD[:], confd[:], AF.Tanh, scale=0.5,
                                 accum_out=_v(partials[:], 7, [[1, 1]]))
            SQD = pool.tile([P, DF], f32, name="SQD")
            nc.scalar.activation(SQD[:], UD[:], AF.Square,
                                 accum_out=_v(partials[:], 8, [[1, 1]]))

            # ---------------- vector engine program
            tcnt = [0]

            def tmp(n, dtype=f32):
                tcnt[0] += 1
                return pool.tile([P, n], dtype, name=f"t{tcnt[0]}")

            # sh = EW * sqrt(anchor) = sqrt(pred_wh) -> SRC q0, q1
            SH = _v(SRC[:], 0, [[TA, 2], [A, T], [1, A]])
            nc.vector.tensor_tensor(
                out=SH,
                in0=_v(EW[:], 0, [[1, 2], [2 * A, T], [2, A]]),
                in1=_v(fp[:], O["SQA"], [[1, 2], [0, T], [2, A]]),
                op=OP.mult)
            # wfull = sh*sh = pred_wh (xi-space half-width), (d, t, a)
            wf = tmp(TA2)
            SH2 = _v(SRC[:], 0, [[TA, 2], [1, TA]])
            WF = _v(wf[:], 0, [[TA, 2], [1, TA]])
            nc.vector.tensor_tensor(out=WF, in0=SH2, in1=SH2, op=OP.mult)

            # IoU in xi coords. XY = SRC q2, q3 as (d, t, a)
            XY = _v(SRC[:], 2 * TA, [[TA, 2], [1, TA]])
            lo = tmp(TA2)
            nc.vector.tensor_tensor(out=lo[:], in0=XY, in1=WF, op=OP.subtract)
            hi = tmp(TA2)
            nc.vector.tensor_tensor(out=hi[:], in0=XY, in1=WF, op=OP.add)
            B1v = _v(fp[:], O["B1"], [[T, 2], [1, T], [0, A]])
            B2v = _v(fp[:], O["B2"], [[T, 2], [1, T], [0, A]])
            LOv = _v(lo[:], 0, [[TA, 2], [A, T], [1, A]])
            HIv = _v(hi[:], 0, [[TA, 2], [A, T], [1, A]])
            t1 = tmp(TA2)
            nc.vector.tensor_tensor(out=_v(t1[:], 0, [[TA, 2], [A, T], [1, A]]),
                                    in0=HIv, in1=B2v, op=OP.min)
            t2 = tmp(TA2)
            nc.vector.tensor_tensor(out=_v(t2[:], 0, [[TA, 2], [A, T], [1, A]]),
                                    in0=LOv, in1=B1v, op=OP.max)
            t3 = tmp(TA2)
            nc.vector.tensor_tensor(out=t3[:], in0=t1[:], in1=t2[:], op=OP.subtract)
            # iw = max(t3, 0) * 0.5 -> physical overlap widths (d, t, a)
            iwih = tmp(TA2)
            nc.vector.tensor_scalar(out=iwih[:], in0=t3[:], scalar1=0.0,
                                    scalar2=0.5, op0=OP.max, op1=OP.mult)
            inter = tmp(TA)
            nc.vector.tensor_tensor(out=inter[:], in0=_v(iwih[:], 0, [[1, TA]]),
                                    in1=_v(iwih[:], TA, [[1, TA]]), op=OP.mult)
            # union side branch on gpsimd, in parallel with the inter chain
            areaA = tmp(TA)
            nc.gpsimd.tensor_tensor(out=areaA[:], in0=_v(wf[:], 0, [[1, TA]]),
                                    in1=_v(wf[:], TA, [[1, TA]]), op=OP.mult)
            u1 = tmp(TA)
            nc.gpsimd.tensor_tensor(out=_v(u1[:], 0, [[A, T], [1, A]]),
                                    in0=_v(areaA[:], 0, [[A, T], [1, A]]),
                                    in1=_v(fp[:], O["TAREA"], [[1, T], [0, A]]),
                                    op=OP.add)
            u2 = tmp(TA)
            nc.vector.tensor_tensor(out=u2[:], in0=u1[:], in1=inter[:],
                                    op=OP.subtract)
            rcp = tmp(TA)
            nc.vector.reciprocal(out=rcp[:], in_=u2[:])
            iou = tmp(TA)
            nc.vector.tensor_tensor(out=iou[:], in0=inter[:], in1=rcp[:],
                                    op=OP.mult)

            # first-argmax over a -> fmask (exact float equality + tiebreak)
            rmax = tmp(T)
            nc.vector.tensor_reduce(out=rmax[:],
                                    in_=_v(iou[:], 0, [[A, T], [1, A]]),
                                    axis=AX.X, op=OP.max)
            eq = tmp(TA)
            nc.vector.tensor_tensor(out=_v(eq[:], 0, [[A, T], [1, A]]),
                                    in0=_v(iou[:], 0, [[A, T], [1, A]]),
                                    in1=_v(rmax[:], 0, [[1, T], [0, A]]),
                                    op=OP.is_equal)
            # WOBJ = (A - a) * obj: padding cells give fval = 0 -> fmask all-1
            # there, which is harmless: u/uu/ce/box are all zero at padding.
            fval = tmp(TA)
            nc.vector.tensor_tensor(out=_v(fval[:], 0, [[A, T], [1, A]]),
                                    in0=_v(eq[:], 0, [[A, T], [1, A]]),
                                    in1=_v(fp[:], O["WCONST"], [[A, T], [1, A]]),
                                    op=OP.mult)
            m2 = tmp(T)
            nc.vector.tensor_reduce(out=m2[:],
                                    in_=_v(fval[:], 0, [[A, T], [1, A]]),
                                    axis=AX.X, op=OP.max)
            fmask = tmp(TA)
            nc.vector.tensor_tensor(out=_v(fmask[:], 0, [[A, T], [1, A]]),
                                    in0=_v(fval[:], 0, [[A, T], [1, A]]),
                                    in1=_v(m2[:], 0, [[1, T], [0, A]]),
                                    op=OP.is_equal)
            # cls path: se = sum_j e (bf16), lse bit trick + ce on gpsimd
            se = tmp(TA, bf16)
            with nc.allow_low_precision(reason="lse bit-trick needs bf16 bits; "
                                        "2e-2 loss tolerance"):
                nc.vector.tensor_reduce(
                    out=_v(se[:], 0, [[1, TA]]),
                    in_=_v(e[:], 0, [[NCLS, TA], [1, NCLS]]),
                    axis=AX.X, op=OP.add)
            lgf = tmp(TA)
            nc.gpsimd.tensor_copy(out=lgf[:], in_=se[:].bitcast(i16))
            lse = tmp(TA)
            nc.gpsimd.tensor_scalar(out=lse[:], in0=lgf[:],
                                    scalar1=LN2 / (1 << 7),
                                    scalar2=-LOG_BIAS * LN2,
                                    op0=OP.mult, op1=OP.add)
            nc.gpsimd.tensor_tensor(out=_v(SRC[:], 6 * TA, [[A, T], [1, A]]),
                                    in0=_v(lse[:], 0, [[A, T], [1, A]]),
                                    in1=_v(fp[:], O["SAUX"], [[A, T], [1, A]]),
                                    op=OP.subtract)

            # best-anchor selection of all seven quantities in one mul+reduce
            selm = pool.tile([P, 7 * TA], f32, name="selm")
            nc.vector.tensor_tensor(out=_v(selm[:], 0, [[TA, 7], [1, TA]]),
                                    in0=_v(SRC[:], 0, [[TA, 7], [1, TA]]),
                                    in1=_v(fmask[:], 0, [[0, 7], [1, TA]]),
                                    op=OP.mult)
            FIN = pool.tile([P, 7 * T], f32, name="FIN")
            nc.vector.tensor_reduce(out=_v(FIN[:], 0, [[T, 7], [1, T]]),
                                    in_=_v(selm[:], 0, [[TA, 7], [A, T], [1, A]]),
                                    axis=AX.X, op=OP.add)

            # box: FIN q0..3 -> oscl4*(sel - aux4)^2, written back into q0..3
            d4 = tmp(4 * T)
            nc.vector.tensor_tensor(out=d4[:], in0=_v(FIN[:], 0, [[1, 4 * T]]),
                                    in1=_v(fp[:], O["AUX4"], [[1, 4 * T]]),
                                    op=OP.subtract)
            d4m = tmp(4 * T)
            nc.vector.tensor_tensor(out=d4m[:], in0=d4[:],
                                    in1=_v(fp[:], O["OSCL4"], [[1, 4 * T]]),
                                    op=OP.mult)
            nc.vector.tensor_tensor(out=_v(FIN[:], 0, [[1, 4 * T]]),
                                    in0=d4[:], in1=d4m[:], op=OP.mult)

            # one grouped reduce -> partials cols 0..6
            nc.vector.tensor_reduce(out=_v(partials[:], 0, [[1, 7]]),
                                    in_=_v(FIN[:], 0, [[T, 7], [1, T]]),
                                    axis=AX.X, op=OP.add)

            nc.sync.dma_start(out=partials_d[:], in_=partials[:])

    if split:
        _split_multi_waits(nc)
    _strip_unused_const_memsets(nc)
    return nc


def _strip_unused_const_memsets(nc):
    """Drop the Bass-prologue memsets for const APs this kernel never reads
    (only const-float32-0.0 is used, as the activation bias)."""
    drop = ("const-float32-1.0", "const-bfloat16-1.0", "const-uint8-127")
    for fn in nc.m.functions:
        for blk in fn.blocks:
            keep = []
            for ins in blk.instructions:
                outs = getattr(ins, "outs", None) or []
                names = {getattr(getattr(o, "tensor", None), "name", "")
                         for o in outs}
                if type(ins).__name__ == "InstMemSet" and names & set(drop):
                    continue
                keep.append(ins)
            blk.instructions = keep


# -------------------------------------------------------------- shard builder
def _make_in_maps(out, gt_boxes, anchor_np, gt_classes_np, num_box_np, T):
    obj, xo, yo, tw, th, cls_t = _build_target_np(gt_boxes, gt_classes_np,
                                                  num_box_np)
    SLOTS = P * T
    TA = T * A
    out_r = out.reshape(B, A, 25, HWC)
    sqa = np.sqrt(anchor_np)                       # [A, 2]

    in_maps = []
    for c in range(CORES):
        sl = slice(c * BC, (c + 1) * BC)
        ob = obj[sl]                               # [BC, HWC]
        bloc, hwloc = np.nonzero(ob > 0)
        K = len(bloc)
        assert K <= SLOTS

        def place(vals):
            buf = np.zeros(SLOTS, dtype=np.float32)
            buf[:K] = vals
            return buf.reshape(P, T)

        objv = place(np.ones(K, dtype=np.float32))
        xov = place(xo[sl][bloc, hwloc])
        yov = place(yo[sl][bloc, hwloc])
        twv = place(tw[sl][bloc, hwloc])
        thv = place(th[sl][bloc, hwloc])

        # occupied-cell prediction channels [K, A, 25]
        colsb = np.zeros((SLOTS, A, 25), dtype=np.float32)
        if K:
            colsb[:K] = out_r[sl].transpose(0, 3, 1, 2)[bloc, hwloc]
        # cols_xw (t, ch{x,y,conf,w,h}, a)
        sel = colsb[:, :, [21, 22, 20, 23, 24]]            # (t, a, ch)
        cols_xw = np.ascontiguousarray(
            sel.reshape(P, T, A, 5).transpose(0, 1, 3, 2)).reshape(P, 25 * T)
        logits = np.ascontiguousarray(
            colsb[:, :, :20]).reshape(P, 100 * T)          # (t, a, j)

        # target-class logit per (t, a); padding slots get the exact device
        # lse of se=20.0 (logits 0 -> e all 1) so padded ce comes out 0
        clsv = place(cls_t[sl][bloc, hwloc].astype(np.float32)).astype(np.int64)
        s_aux = np.take_along_axis(
            colsb[:, :, :20].reshape(SLOTS, A, 20),
            clsv.reshape(SLOTS, 1, 1).repeat(A, axis=1), axis=2
        )[:, :, 0].astype(np.float32)
        i16pad = float(_bf16(np.float32(20.0)).view(np.int16))
        lse_pad = np.float32(np.float32(i16pad * np.float32(LN2 / (1 << 7)))
                             + np.float32(-LOG_BIAS * LN2))
        s_aux[K:] = lse_pad
        s_aux = s_aux.reshape(P, TA)

        # xi-space target box edges (d{x,y}, t): center 2o-1, half-width t_wh
        cxv = 2.0 * xov - 1.0
        cyv = 2.0 * yov - 1.0
        b1 = np.stack([cxv - twv, cyv - thv], axis=1).reshape(P, 2 * T)
        b2 = np.stack([cxv + twv, cyv + thv], axis=1).reshape(P, 2 * T)
        tarea = (twv * thv).reshape(P, T)

        # (A - a) * obj in (t, a) layout: padding cells get all-zero fval
        wconst = np.ascontiguousarray(
            (A - np.arange(A, dtype=np.float32))[None, None, :]
            * objv[:, :, None]).reshape(P, 5 * T)
        sqav = np.broadcast_to(sqa.reshape(1, 10), (P, 10))

        # AUX4 / OSCL4 in q-order (w, h, x, y)
        aux4 = np.stack([np.sqrt(twv), np.sqrt(thv), cxv, cyv],
                        axis=1).reshape(P, 4 * T)
        oscl4 = np.stack([objv, objv, 0.25 * objv, 0.25 * objv],
                         axis=1).reshape(P, 4 * T)

        fpack = np.concatenate(
            [cols_xw, b1, b2, tarea, wconst, sqav,
             s_aux, aux4, oscl4, objv.reshape(P, T)], axis=1)

        # dense conf channels: [BC, A, HWC] -> [P, 1280] bf16
        confd = out_r[sl][:, :, 20, :].reshape(P, -1)

        in_maps.append({
            "fpack": np.ascontiguousarray(fpack, dtype=np.float32),
            "lgpack": _bf16(logits),
            "confd": _bf16(confd),
        })
    return in_maps


# ---------------------------------------------------------------- entry point
def kernel(out, gt_boxes, anchor, gt_classes, num_box):
    from concourse.bass_utils import run_bass_kernel_spmd

    out = np.ascontiguousarray(np.asarray(out, dtype=np.float32))
    gt_boxes = np.asarray(gt_boxes, dtype=np.float32)
    anchor_np = np.asarray(anchor, dtype=np.float32)
    gt_classes_np = np.asarray(gt_classes)
    num_box_np = np.asarray(num_box)

    # per-core occupied-cell counts decide the compiled tile factor T
    obj = _build_target_np(gt_boxes, gt_classes_np, num_box_np)[0]
    ks = [int((obj[c * BC:(c + 1) * BC] > 0).sum()) for c in range(CORES)]
    maxk = max(ks)
    T = max(1, -(-maxk // P))
    assert maxk <= 13 * P and T <= 13

    in_maps = _make_in_maps(out, gt_boxes, anchor_np, gt_classes_np,
                            num_box_np, T)

    import os
    key = f"nc{T}"
    if key not in _CACHE:
        _CACHE[key] = _build_nc(T)
    trace = os.environ.get("KERNEL_TRACE", "0") == "1"
    res = run_bass_kernel_spmd(_CACHE[key], in_maps, core_ids=list(range(CORES)),
                               trace=trace)
    if trace:
        print(f"HW exec time: {res.exec_time_ns} ns  (mean {res.mean_exec_time_ns})")

    cols = np.zeros(12, dtype=np.float64)
    for c in range(CORES):
        cols += res.results[c]["partials"].astype(np.float64).sum(axis=0)
    K = float(sum(ks))
    box_loss = np.float32(LAM_COORD / B * (cols[0] + cols[1] + cols[2] + cols[3]))
    conf_loss = np.float32(LAM_OBJ / B * 0.25 * (cols[5] - 2.0 * cols[4] + K))
    nob_c = 0.25 * (cols[5] + 2.0 * cols[4] + K)
    dense = 0.25 * (float(B * A * HWC) + 2.0 * cols[7] + cols[8])
    noobj_loss = np.float32(LAM_NOOBJ / B * (dense - nob_c))
    cls_loss = np.float32(LAM_CLS / B * cols[6])
    return (box_loss, conf_loss, noobj_loss, cls_loss)
